# revision 50
# baseline (speedup 1.0000x reference)
"""Trainium2 Bass kernel for nn_CTCPerSpeakerExtractorConcatNNG.

Sharding: 8 cores = (batch b, speaker k) pairs; each core runs the full
T=1536 stream for its pair. No collectives; host scatters/gathers.

Per-core dataflow (natural layout [T-tiles x 128 part, D free], bf16 acts):
  X = xmT.T @ Win + bin               (xmT pre-transposed on host)
  LN_kv(X) -> transpose -> KVT -> KT (transposed), V (halo-tiled natural)
  Xk = X * sigmoid(6(A-.5));  LN_q -> transpose -> QT (transposed)
  banded attention (BAND=24) with 128-query tiles x 176-key windows
  y2 = Xk + attn@Wo ; LN_f -> transpose -> FFN (gelu) ; y3 = y2 + h2 + b2k
  out = LN_s(y3) normalized only; host applies ln_s gain/bias.
LN gains/biases for kv/q/f are folded into the following matmul on host.
"""
import sys

for _p in ("/opt/trn_rl_repo", "/root/.axon_site/_ro/trn_rl_repo"):
    if _p not in sys.path:
        sys.path.append(_p)

from contextlib import ExitStack

import numpy as np
import ml_dtypes

import concourse.bass as bass
import concourse.bacc as bacc
import concourse.tile as tile
from concourse import mybir
from concourse.bass_utils import run_bass_kernel_spmd
from concourse.masks import make_identity

BF = mybir.dt.bfloat16
F32 = mybir.dt.float32
I32 = mybir.dt.int32
AF = mybir.ActivationFunctionType
OP = mybir.AluOpType
MAGIC1 = 0x5F3759DF + 1

B, T, D, KSP, H, BAND = 4, 1536, 512, 2, 8, 24
DH = D // H          # 64
P = 128
NT = T // P          # 12
WIN = P + 2 * BAND   # 176
NC_D = D // P        # 4 chunks of contraction dim
DFF = 4 * D          # 2048
NDH = DFF // P       # 16
EPS = 1e-5

# V halo-tile starts (each tile = up to 128 rows starting at s)
_VSTARTS = sorted({0, 128, 1360, 1488} | {128 * m - 24 for m in range(1, 12)})
_VIDX = {s: j for j, s in enumerate(_VSTARTS)}
NV = len(_VSTARTS)   # 15


def _bcast_ap(dram_ap, parts=128):
    """[N] dram vector -> [parts, N] broadcast AP (partition step 0)."""
    return bass.AP(
        tensor=dram_ap.tensor,
        offset=dram_ap.offset,
        ap=[[0, parts]] + list(dram_ap.ap),
    )


def _ln_stats_into(nc, pool, in_ap, mv_out):
    """bn_stats/bn_aggr for one tile; (mean, var) land in mv_out [128, 2]."""
    st = pool.tile([P, 6], F32, tag="bn_st")
    nc.vector.bn_stats(out=st, in_=in_ap)
    nc.vector.bn_aggr(out=mv_out, in_=st)


def _rsqrt_cols(nc, pool, v_ap, out_ap, n):
    """out = (v + EPS)^-0.5 for [128, n] columns, DVE only (no act table).

    Quake-style seed: bits exact via shift+xor; the +MAGIC add runs through
    the DVE's fp32 ALU (rounds above 2^24), which only perturbs low mantissa
    seed bits. Two Newton iterations finish to ~5e-6 rel err."""
    vp = pool.tile([P, n], F32, tag="rs_vp")
    nc.vector.tensor_scalar_add(out=vp, in0=v_ap, scalar1=EPS)
    nh = pool.tile([P, n], I32, tag="rs_nh")
    nc.vector.tensor_scalar(out=nh, in0=vp[:, :].bitcast(I32), scalar1=1,
                            scalar2=-1, op0=OP.logical_shift_right,
                            op1=OP.bitwise_xor)
    y0i = pool.tile([P, n], I32, tag="rs_y0")
    nc.vector.tensor_scalar_add(out=y0i, in0=nh, scalar1=MAGIC1)
    y = y0i[:, :].bitcast(F32)
    for it in range(2):
        t1 = pool.tile([P, n], F32, tag=f"rs_t{it}")
        nc.vector.tensor_tensor(out=t1, in0=y, in1=y, op=OP.mult)
        nc.vector.tensor_tensor(out=t1, in0=t1, in1=vp, op=OP.mult)
        nc.vector.tensor_scalar(out=t1, in0=t1, scalar1=-0.5, scalar2=1.5,
                                op0=OP.mult, op1=OP.add)
        if it == 0:
            yn = pool.tile([P, n], F32, tag="rs_yn")
            nc.vector.tensor_tensor(out=yn, in0=t1, in1=y, op=OP.mult)
            y = yn
        else:
            nc.vector.tensor_tensor(out=out_ap, in0=t1, in1=y, op=OP.mult)


def build_program(add_bo: bool, stop_stage: int = 99, add_bin: bool = False,
                  add_bv: bool = False) -> bass.Bass:
    nc = bacc.Bacc()

    # ---- DRAM I/O ----
    xmT = nc.dram_tensor("xmT", [D, T], BF, kind="ExternalInput")
    Wd = {}
    for nm, (di, do) in [("Win", (D, D)), ("Wq", (D, D)), ("Wk", (D, D)),
                         ("Wv", (D, D)), ("Wo", (D, D)), ("W1", (D, DFF)),
                         ("W2", (DFF, D))]:
        Wd[nm] = nc.dram_tensor(nm, [di, do], BF, kind="ExternalInput")
    smalls_d = nc.dram_tensor("smalls", [P, 36], F32, kind="ExternalInput")
    rows_d = nc.dram_tensor("rows", [4, D], F32, kind="ExternalInput")
    out_d = nc.dram_tensor("out", [T, D], F32, kind="ExternalOutput")
    var_d = nc.dram_tensor("var", [P, NT], F32, kind="ExternalOutput")
    out_t = out_d.rearrange("(n p) d -> n p d", p=P)

    with tile.TileContext(nc) as tc, ExitStack() as ctx:
        def _body():
            consts = ctx.enter_context(tc.tile_pool(name="consts", bufs=1))
            wpool = ctx.enter_context(tc.tile_pool(name="wpool", bufs=1))
            acts = ctx.enter_context(tc.tile_pool(name="acts", bufs=1))
            ln_nat_p = ctx.enter_context(tc.tile_pool(name="ln_nat_p", bufs=2))
            tT_p = ctx.enter_context(tc.tile_pool(name="tT_p", bufs=1))
            kqh = ctx.enter_context(tc.tile_pool(name="kqh", bufs=1))
            kv_p = ctx.enter_context(tc.tile_pool(name="kv_p", bufs=1))
            small = ctx.enter_context(tc.tile_pool(name="small", bufs=6))
            sm2 = ctx.enter_context(tc.tile_pool(name="sm2", bufs=2))
            outp = ctx.enter_context(tc.tile_pool(name="outp", bufs=2))
            psA = ctx.enter_context(tc.tile_pool(name="psA", bufs=2, space="PSUM"))
            psT = ctx.enter_context(tc.tile_pool(name="psT", bufs=1, space="PSUM"))
            psB = ctx.enter_context(tc.tile_pool(name="psB", bufs=3, space="PSUM"))
            psD = ctx.enter_context(tc.tile_pool(name="psD", bufs=1, space="PSUM"))
            psE = ctx.enter_context(tc.tile_pool(name="psE", bufs=1, space="PSUM"))

            def dbg_out(get_tile_view, grouped=False):
                # copy 12 [128,512] views (cast to f32) to out and stop
                for _mt in range(NT):
                    o = outp.tile([P, D], F32, tag="o_sb")
                    ov = o.rearrange("p (c q) -> p c q", c=NC_D) if grouped else o
                    nc.vector.tensor_copy(out=ov, in_=get_tile_view(_mt))
                    nc.sync.dma_start(out=out_t[_mt], in_=o)

            # ---- constants ----
            ident = consts.tile([P, P], BF)
            make_identity(nc, ident)
            eps_t = consts.tile([P, 1], F32, tag="eps_t")
            nc.vector.memset(eps_t, EPS)
            pos3_t = consts.tile([P, 1], F32, tag="pos3_t")
            nc.vector.memset(pos3_t, 3.0)
            ones_c = consts.tile([P, 1], BF, tag="ones_c")
            nc.vector.memset(ones_c, 1.0)
            var_sb = consts.tile([P, NT], F32, tag="var_sb")
            # Transposed band masks: mk[j, q] (j = key index in window on the
            # partition axis, q = query on the free axis), in two chunks
            # (A: j 0..127, B: j 128..175 on partitions 0..47).
            # keep iff 0 <= (j - q - off) <= 2*BAND.
            # masks[nm] layout [128, 2, 128]: chunk A ([:, 0, :], key j =
            # partition) and chunk B for even heads ([0:48, 1, :], key j =
            # 128 + partition). masksB64[nm] [128, 128] holds the chunk-B
            # mask on partitions 64:112 (key j = 128 + p - 64) so the odd
            # heads' mask-add matmul can keep its lhsT at partition offset
            # 64 — an accumulation group whose matmuls mix lhsT partition
            # offsets (64 then 0) at M<128 wedges the PE at runtime.
            masks, masksB64 = {}, {}
            NEG = -1e30
            for off, nm in ((0, "mid"), (-BAND, "first"), (BAND, "last")):
                mk = consts.tile([P, 2, P], BF, tag=f"mask_{nm}")
                nc.gpsimd.memset(mk, 0.0)
                for ch, jo in ((0, 0), (1, P)):
                    v = mk[:, ch, :]
                    # keep iff (j - q - off) >= 0 else -inf
                    nc.gpsimd.affine_select(
                        out=v, in_=v, compare_op=OP.is_ge, fill=NEG,
                        base=jo - off, pattern=[[-1, P]], channel_multiplier=1)
                    # keep iff (q + off + 2*BAND - j) >= 0 else -inf
                    nc.gpsimd.affine_select(
                        out=v, in_=v, compare_op=OP.is_ge, fill=NEG,
                        base=off + 2 * BAND - jo, pattern=[[1, P]],
                        channel_multiplier=-1)
                masks[nm] = mk
                mb = consts.tile([P, P], BF, tag=f"maskB64_{nm}")
                nc.gpsimd.memset(mb, 0.0)
                # key j = 128 + p - 64: keep iff 0 <= (j - q - off) <= 2*BAND
                nc.gpsimd.affine_select(
                    out=mb, in_=mb, compare_op=OP.is_ge, fill=NEG,
                    base=64 - off, pattern=[[-1, P]], channel_multiplier=1)
                nc.gpsimd.affine_select(
                    out=mb, in_=mb, compare_op=OP.is_ge, fill=NEG,
                    base=off + 2 * BAND - 64, pattern=[[1, P]],
                    channel_multiplier=-1)
                masksB64[nm] = mb
            # identity block at partition offset 64: ident_s64[p, j] = 1 iff
            # p == j + 64 (rows 64:112 x cols 0:48 form I_48)
            ident_s64 = consts.tile([P, 48], BF, tag="ident_s64")
            nc.gpsimd.memset(ident_s64, 1.0)
            nc.gpsimd.affine_select(
                out=ident_s64, in_=ident_s64, compare_op=OP.is_equal, fill=0.0,
                base=-64, pattern=[[-1, 48]], channel_multiplier=1)

            def mask_name(mt):
                return "first" if mt == 0 else ("last" if mt == NT - 1 else "mid")

            def ws_of(mt):
                return min(max(mt * P - BAND, 0), T - WIN)

            # packed small tensors first: they gate What/Xk and the whole X
            # pipeline, and cost ~0.5us vs ~20us of weight DMA ahead of them.
            # cols [0:12]=A_k tiled, [12:16]=bq4, [16:20]=bk4, [20:36]=b1_16
            smalls = consts.tile([P, 36], F32, tag="smalls")
            nc.sync.dma_start(out=smalls, in_=smalls_d[:, :])

            xmT_s = ln_nat_p.tile([P, NC_D, T], BF, tag="ln_nat")
            nc.sync.dma_start(out=xmT_s, in_=xmT.rearrange("(c p) t -> p c t", p=P))

            # ---- weights/biases to SBUF (order: earliest-needed first).
            # W1/W2 (4MB, needed only at the FFN) go through the Pool queue so
            # they stream in parallel with SP's projection weights.
            # Tag sharing reuses dead weight buffers: Wo lands in Win's spot
            # once the X stage drained it; W1/W2 land in Wk/Wq's (their DMAs
            # ride the idle Pool queue and block on the tile-free semaphore).
            Ws = {}
            for nm, tag in (("Win", "WinWo"), ("Wk", "WkW1"), ("Wv", "Wv"),
                            ("Wq", "WqW2")):
                di, do = Wd[nm].shape
                t = wpool.tile([P, di // P, do], BF, tag=tag)
                nc.sync.dma_start(out=t, in_=Wd[nm].rearrange("(c p) o -> p c o", p=P))
                Ws[nm] = t
            t = wpool.tile([P, NC_D, D], BF, tag="WinWo")
            nc.sync.dma_start(out=t, in_=Wd["Wo"].rearrange("(c p) o -> p c o", p=P))
            Ws["Wo"] = t
            t = wpool.tile([P, NC_D, DFF], BF, tag="WkW1")
            nc.gpsimd.dma_start(out=t, in_=Wd["W1"].rearrange("(c p) o -> p c o", p=P))
            Ws["W1"] = t
            W2s = wpool.tile([P, NDH, D], BF, tag="WqW2")
            nc.gpsimd.dma_start(out=W2s, in_=Wd["W2"].rearrange("(c p) o -> p c o", p=P))
            Ak_s = smalls[:, 0:NT]
            bq4 = smalls[:, 12:12 + NC_D]
            bk4 = smalls[:, 16:16 + NC_D]
            b1_16 = smalls[:, 20:20 + NDH]
            # bias row broadcasts (partition-step-0 DMA reads, contiguous source)
            if add_bin:
                bin_b = consts.tile([P, D], F32, tag="bin_b")
                nc.sync.dma_start(out=bin_b, in_=_bcast_ap(rows_d[0, :]))
            if add_bv:
                bv_b = consts.tile([P, D], F32, tag="bv_b")
                nc.sync.dma_start(out=bv_b, in_=_bcast_ap(rows_d[1, :]))
            b2k_b = consts.tile([P, D], F32, tag="b2k_b")
            nc.sync.dma_start(out=b2k_b, in_=_bcast_ap(rows_d[2, :]))
            if add_bo:
                ones_r = consts.tile([1, P], BF, tag="ones_r")
                nc.vector.memset(ones_r, 1.0)
                bo_rf = consts.tile([1, D], F32, tag="bo_rf")
                nc.sync.dma_start(out=bo_rf, in_=rows_d[3:4, :])
                bo_rb = consts.tile([1, D], BF, tag="bo_rb")
                nc.vector.tensor_copy(out=bo_rb, in_=bo_rf)

            # sigmoid(6(A-.5)) = 1/(1+exp(-6A+3)) — stays in the Exp act table
            What_e = consts.tile([P, NT], F32, tag="What_e")
            nc.scalar.activation(out=What_e, in_=Ak_s, func=AF.Exp, scale=-6.0,
                                 bias=pos3_t)
            nc.vector.tensor_scalar_add(out=What_e, in0=What_e, scalar1=1.0)
            What = consts.tile([P, NT], F32, tag="What")
            nc.vector.reciprocal(out=What, in_=What_e)

            # ---- X = xmT.T @ Win + bin ; LN_kv ; Xk (X stays in PSUM only) ----
            # Only Xk = X * What leaves the PSUM; LN_kv is reconstructed from
            # Xk as lnkv = Xk * (rstd/What) - mean*rstd (relative bf16 error
            # is unchanged by the gate divide). rstd comes from the DVE-only
            # Newton rsqrt, batched per 4-tile group so the Activation engine
            # runs no LN work (and no Sqrt/Ln act-table loads, ever).
            rWhat = consts.tile([P, NT], F32, tag="rWhat")
            nc.vector.reciprocal(out=rWhat, in_=What)
            mv_kv = consts.tile([P, 2, NT], F32, tag="mv_kv")
            rstd_kv = consts.tile([P, NT], F32, tag="rstd_kv")
            lnkv = ln_nat_p.tile([P, NT, D], BF, tag="ln_nat")
            Xk = acts.tile([P, NT, D], BF, tag="Xk")
            for g in range(NT // 4):
                for i in range(4):
                    mt = 4 * g + i
                    ps = psA.tile([P, D], F32, tag="psA")
                    for c in range(NC_D):
                        nc.tensor.matmul(
                            ps, lhsT=xmT_s[:, c, mt * P:(mt + 1) * P],
                            rhs=Ws["Win"][:, c, :],
                            start=(c == 0), stop=(c == NC_D - 1))
                    if add_bin:
                        psb = sm2.tile([P, D], F32, tag="Xpsb")
                        nc.vector.tensor_tensor(out=psb, in0=ps, in1=bin_b, op=OP.add)
                    else:
                        psb = ps
                    _ln_stats_into(nc, small, psb, mv_kv[:, :, mt])
                    nc.scalar.activation(out=Xk[:, mt, :], in_=psb, func=AF.Copy,
                                         scale=What[:, mt:mt + 1])
                _rsqrt_cols(nc, small, mv_kv[:, 1, 4 * g:4 * g + 4],
                            rstd_kv[:, 4 * g:4 * g + 4], 4)
                for i in range(4):
                    mt = 4 * g + i
                    s1 = small.tile([P, 1], F32, tag="ln_s1")
                    nc.vector.tensor_tensor(out=s1, in0=rstd_kv[:, mt:mt + 1],
                                            in1=rWhat[:, mt:mt + 1], op=OP.mult)
                    s2 = small.tile([P, 1], F32, tag="ln_s2")
                    nc.vector.tensor_tensor(out=s2, in0=mv_kv[:, 0, mt:mt + 1],
                                            in1=rstd_kv[:, mt:mt + 1], op=OP.mult)
                    nc.vector.tensor_scalar(out=lnkv[:, mt, :], in0=Xk[:, mt, :],
                                            scalar1=s1, scalar2=s2,
                                            op0=OP.mult, op1=OP.subtract)

            if stop_stage == 1:
                dbg_out(lambda m: lnkv[:, m, :])
                return

            # ---- transpose LN_kv -> KVT [128, 4, T] ----
            def transpose_nat_to_T(src, dst):
                for mt in range(NT):
                    pt = psT.tile([P, NC_D * P], BF, tag="psT")
                    for c in range(NC_D):
                        nc.tensor.transpose(
                            pt[:, c * P:(c + 1) * P], src[:, mt, c * P:(c + 1) * P], ident)
                    nc.vector.tensor_copy(
                        out=dst[:, :, mt * P:(mt + 1) * P],
                        in_=pt.rearrange("p (c q) -> p c q", c=NC_D))

            KVT = tT_p.tile([P, NC_D, T], BF, tag="tT")
            transpose_nat_to_T(lnkv, KVT)

            if stop_stage == 2:
                dbg_out(lambda m: KVT[:, :, m * P:(m + 1) * P], grouped=True)
                return

            # ---- KT (transposed) and V (halo natural) ----
            KT = kv_p.tile([P, NC_D, T], BF, tag="KT")
            for co in range(NC_D):
                for tch in range(3):
                    ps = psA.tile([P, D], F32, tag="psA")
                    for c in range(NC_D):
                        nc.tensor.matmul(
                            ps, lhsT=Ws["Wk"][:, c, co * P:(co + 1) * P],
                            rhs=KVT[:, c, tch * D:(tch + 1) * D],
                            start=(c == 0), stop=(c == NC_D - 1))
                    nc.scalar.activation(
                        out=KT[:, co, tch * D:(tch + 1) * D], in_=ps,
                        func=AF.Identity, bias=bk4[:, co:co + 1])

            Vh = acts.tile([P, NV, D], BF, tag="Vh")
            for j, s in enumerate(_VSTARTS):
                w = min(P, T - s)
                ps = psA.tile([P, D], F32, tag="psA")
                for c in range(NC_D):
                    nc.tensor.matmul(
                        ps[0:w, :], lhsT=KVT[:, c, s:s + w], rhs=Ws["Wv"][:, c, :],
                        start=(c == 0), stop=(c == NC_D - 1))
                if add_bv:
                    nc.vector.tensor_tensor(
                        out=Vh[0:w, j, :], in0=ps[0:w, :], in1=bv_b[0:w, :], op=OP.add)
                else:
                    nc.scalar.copy(out=Vh[0:w, j, :], in_=ps[0:w, :])

            if stop_stage == 3:
                dbg_out(lambda m: Vh[:, m, :])
                return

            # ---- LN_q on Xk ; transpose ; QT ----
            mv_q = consts.tile([P, 2, NT], F32, tag="mv_q")
            rstd_q = consts.tile([P, NT], F32, tag="rstd_q")
            lnq = ln_nat_p.tile([P, NT, D], BF, tag="ln_nat")
            for g in range(NT // 4):
                for i in range(4):
                    mt = 4 * g + i
                    _ln_stats_into(nc, small, Xk[:, mt, :], mv_q[:, :, mt])
                _rsqrt_cols(nc, small, mv_q[:, 1, 4 * g:4 * g + 4],
                            rstd_q[:, 4 * g:4 * g + 4], 4)
                for i in range(4):
                    mt = 4 * g + i
                    nc.vector.tensor_scalar(out=lnq[:, mt, :], in0=Xk[:, mt, :],
                                            scalar1=mv_q[:, 0, mt:mt + 1],
                                            scalar2=rstd_q[:, mt:mt + 1],
                                            op0=OP.subtract, op1=OP.mult)
            LNQT = tT_p.tile([P, NC_D, T], BF, tag="tT")
            transpose_nat_to_T(lnq, LNQT)

            QT = kv_p.tile([P, NC_D, T], BF, tag="QT")
            for co in range(NC_D):
                for tch in range(3):
                    ps = psA.tile([P, D], F32, tag="psA")
                    for c in range(NC_D):
                        nc.tensor.matmul(
                            ps, lhsT=Ws["Wq"][:, c, co * P:(co + 1) * P],
                            rhs=LNQT[:, c, tch * D:(tch + 1) * D],
                            start=(c == 0), stop=(c == NC_D - 1))
                    nc.scalar.activation(
                        out=QT[:, co, tch * D:(tch + 1) * D], in_=ps,
                        func=AF.Identity, bias=bq4[:, co:co + 1])

            if stop_stage == 4:
                dbg_out(lambda m: QT[:, :, m * P:(m + 1) * P], grouped=True)
                return

            # ---- attention (transposed scores) + inline Wo/residual/LN_f ----
            # Scores are computed transposed: sT[j, q] = k_j . q_q, two window
            # chunks (A: keys ws..ws+127, B: ws+128..ws+175 on partitions
            # 0..47). exp(sT) is then directly the lhsT of the AV matmul
            # (out natural [q, d]) — no probability transposes or PSUM->SBUF
            # prob copies. Denominators come from N=1 matmuls against a ones
            # column; normalization happens after AV with per-row scalars.
            # Wo + residual + LN_f stats run per tile right after the
            # transpose, so no full-T attention output is ever stored.
            mv_f = consts.tile([P, 2, NT], F32, tag="mv_f")
            rstd_f = consts.tile([P, NT], F32, tag="rstd_f")
            y2 = acts.tile([P, NT, D], BF, tag="y2")
            lnf = ln_nat_p.tile([P, NT, D], BF, tag="ln_nat")
            inv_sqrt_dh = 1.0 / float(np.sqrt(DH))
            attn_tiles = [0] if stop_stage in (41, 42, 43) else range(NT)
            for mt in attn_tiles:
                ws = ws_of(mt)
                mk = masks[mask_name(mt)]
                mkB64 = masksB64[mask_name(mt)]
                if mt == 0:
                    j1, j2 = _VIDX[0], _VIDX[128]
                elif mt == NT - 1:
                    j1, j2 = _VIDX[1360], _VIDX[1488]
                else:
                    j1, j2 = _VIDX[128 * mt - 24], _VIDX[128 * mt + 104]
                pT = sm2.tile([P, H, 2 * P], BF, tag="pT")
                for hpair in range(H // 2):
                    ps = psB.tile([P, 2, 2 * P], F32, tag="psB")
                    for hs in range(2):
                        h = 2 * hpair + hs
                        hp, hc = 64 * (h % 2), h // 2
                        # chunk A: 128 keys x 128 queries (mixed lhsT offsets
                        # are fine at M=128)
                        nc.tensor.matmul(
                            ps[:, hs, 0:P],
                            lhsT=KT[hp:hp + 64, hc, ws:ws + P],
                            rhs=QT[hp:hp + 64, hc, mt * P:(mt + 1) * P],
                            start=True, stop=False)
                        nc.tensor.matmul(ps[:, hs, 0:P], lhsT=ident,
                                         rhs=mk[:, 0, :], start=False, stop=True)
                        # chunk B: 48 keys x 128 queries. The mask-add lhsT
                        # must match the QK lhsT's partition offset (M<128
                        # groups with mixed offsets wedge the PE).
                        nc.tensor.matmul(
                            ps[0:48, hs, P:2 * P],
                            lhsT=KT[hp:hp + 64, hc, ws + P:ws + WIN],
                            rhs=QT[hp:hp + 64, hc, mt * P:(mt + 1) * P],
                            start=True, stop=False)
                        if hp == 0:
                            nc.tensor.matmul(
                                ps[0:48, hs, P:2 * P], lhsT=ident[0:48, 0:48],
                                rhs=mk[0:48, 1, :], start=False, stop=True)
                        else:
                            nc.tensor.matmul(
                                ps[0:48, hs, P:2 * P],
                                lhsT=ident_s64[64:112, :],
                                rhs=mkB64[64:112, :], start=False, stop=True)
                    # batched exp per chunk (partitions 48.. of the B chunks
                    # are uninitialized PSUM and are never read)
                    nc.scalar.activation(
                        out=pT[:, 2 * hpair:2 * hpair + 2, 0:P],
                        in_=ps[:, :, 0:P], func=AF.Exp, scale=inv_sqrt_dh)
                    nc.scalar.activation(
                        out=pT[0:48, 2 * hpair:2 * hpair + 2, P:2 * P],
                        in_=ps[0:48, :, P:2 * P], func=AF.Exp,
                        scale=inv_sqrt_dh)
                if stop_stage == 41:
                    dbg_out(lambda m: pT[:, 0:2, :].rearrange("p h w -> p (h w)"))
                    return
                psy = psD.tile([P, D], F32, tag="psD")
                psden = psE.tile([P, H], F32, tag="psE")
                for h in range(H):
                    nc.tensor.matmul(
                        psy[:, h * DH:(h + 1) * DH], lhsT=pT[:, h, 0:P],
                        rhs=Vh[:, j1, h * DH:(h + 1) * DH],
                        start=True, stop=False)
                    nc.tensor.matmul(
                        psy[:, h * DH:(h + 1) * DH], lhsT=pT[0:48, h, P:2 * P],
                        rhs=Vh[0:48, j2, h * DH:(h + 1) * DH],
                        start=False, stop=True)
                    nc.tensor.matmul(
                        psden[:, h:h + 1], lhsT=pT[:, h, 0:P],
                        rhs=ones_c[:, 0:1], start=True, stop=False)
                    nc.tensor.matmul(
                        psden[:, h:h + 1], lhsT=pT[0:48, h, P:2 * P],
                        rhs=ones_c[0:48, 0:1], start=False, stop=True)
                if stop_stage == 42:
                    dbg_out(lambda m: psy)
                    return
                rden = small.tile([P, H], F32, tag="rden")
                nc.vector.reciprocal(out=rden, in_=psden)
                y_nat = sm2.tile([P, D], BF, tag="y_nat")
                for h in range(H):
                    nc.vector.tensor_scalar_mul(
                        out=y_nat[:, h * DH:(h + 1) * DH],
                        in0=psy[:, h * DH:(h + 1) * DH],
                        scalar1=rden[:, h:h + 1])
                if stop_stage == 43:
                    dbg_out(lambda m: y_nat)
                    return
                ptp = psT.tile([P, NC_D * P], BF, tag="psT")
                for c in range(NC_D):
                    nc.tensor.transpose(
                        ptp[:, c * P:(c + 1) * P], y_nat[:, c * P:(c + 1) * P],
                        ident)
                YTt = sm2.tile([P, NC_D * P], BF, tag="YTt")
                nc.vector.tensor_copy(out=YTt, in_=ptp)
                # Wo + residual for this tile
                ps = psA.tile([P, D], F32, tag="psA")
                for c in range(NC_D):
                    nc.tensor.matmul(
                        ps, lhsT=YTt[:, c * P:(c + 1) * P], rhs=Ws["Wo"][:, c, :],
                        start=(c == 0), stop=(c == NC_D - 1 and not add_bo))
                if add_bo:
                    nc.tensor.matmul(ps, lhsT=ones_r[:, 0:P], rhs=bo_rb,
                                     start=False, stop=True)
                nc.vector.tensor_tensor(
                    out=y2[:, mt, :], in0=ps, in1=Xk[:, mt, :], op=OP.add)
                _ln_stats_into(nc, small, y2[:, mt, :], mv_f[:, :, mt])
                if mt % 4 == 3:
                    g = mt // 4
                    _rsqrt_cols(nc, small, mv_f[:, 1, 4 * g:4 * g + 4],
                                rstd_f[:, 4 * g:4 * g + 4], 4)
                    for i in range(4):
                        m2 = 4 * g + i
                        nc.vector.tensor_scalar(
                            out=lnf[:, m2, :], in0=y2[:, m2, :],
                            scalar1=mv_f[:, 0, m2:m2 + 1],
                            scalar2=rstd_f[:, m2:m2 + 1],
                            op0=OP.subtract, op1=OP.mult)

            if stop_stage == 6:
                dbg_out(lambda m: lnf[:, m, :])
                return

            LNFT = tT_p.tile([P, NC_D, T], BF, tag="tT")
            transpose_nat_to_T(lnf, LNFT)

            # ---- FFN + final residual + LN_s + output ----
            for tch in range(3):
                H1g = kqh.tile([P, NDH, D], BF, tag="kqh")
                for dh in range(NDH):
                    ps = psA.tile([P, D], F32, tag="psA")
                    for c in range(NC_D):
                        nc.tensor.matmul(
                            ps, lhsT=Ws["W1"][:, c, dh * P:(dh + 1) * P],
                            rhs=LNFT[:, c, tch * D:(tch + 1) * D],
                            start=(c == 0), stop=(c == NC_D - 1))
                    nc.scalar.activation(out=H1g[:, dh, :], in_=ps, func=AF.Gelu,
                                         bias=b1_16[:, dh:dh + 1])
                for sub in range(4):
                    mtg = tch * 4 + sub
                    ps = psA.tile([P, D], F32, tag="psA")
                    for dh in range(NDH):
                        nc.tensor.matmul(
                            ps, lhsT=H1g[:, dh, sub * P:(sub + 1) * P],
                            rhs=W2s[:, dh, :], start=(dh == 0), stop=(dh == NDH - 1))
                    y3 = outp.tile([P, D], F32, tag="y3")
                    nc.vector.tensor_tensor(out=y3, in0=ps, in1=y2[:, mtg, :], op=OP.add)
                    y3b = y3
                    nc.gpsimd.tensor_tensor(out=y3b, in0=y3, in1=b2k_b, op=OP.add)
                    # final LN_s: only subtract the mean on device; the
                    # variance is shipped out and the host divides by
                    # sqrt(var+eps) (and applies gain/bias) — keeps Sqrt (a
                    # conflicting act table) out of the Gelu region.
                    st = small.tile([P, 6], F32)
                    nc.vector.bn_stats(out=st, in_=y3b)
                    mv = small.tile([P, 2], F32)
                    nc.vector.bn_aggr(out=mv, in_=st)
                    nc.vector.tensor_copy(out=var_sb[:, mtg:mtg + 1],
                                          in_=mv[:, 1:2])
                    o_sb = outp.tile([P, D], F32, tag="o_sb")
                    nc.vector.tensor_scalar_sub(out=o_sb, in0=y3b,
                                                scalar1=mv[:, 0:1])
                    nc.sync.dma_start(out=out_t[mtg], in_=o_sb)
            nc.sync.dma_start(out=var_d[:, :], in_=var_sb)

        _body()
    nc.finalize()
    return nc


_PROG_CACHE = {}


def kernel(**inputs) -> np.ndarray:
    f32 = np.float32
    bf = ml_dtypes.bfloat16
    x_m = np.asarray(inputs["x_m"], f32)
    A = np.asarray(inputs["A"], f32)
    g = {k: np.asarray(v, f32) for k, v in inputs.items()}

    # fold LN affine params into following matmuls (exact algebra)
    Wq = g["ln_q_g"][:, None] * g["Wq"]
    bq = g["bq"] + g["ln_q_b"] @ g["Wq"]
    Wk = g["ln_kv_g"][:, None] * g["Wk"]
    bk = g["bk"] + g["ln_kv_b"] @ g["Wk"]
    Wv = g["ln_kv_g"][:, None] * g["Wv"]
    bv = g["bv"] + g["ln_kv_b"] @ g["Wv"]
    W1 = g["ln_f_g"][:, None] * g["W1"]
    b1 = g["b1"] + g["ln_f_b"] @ g["W1"]

    add_bo = bool(np.any(g["bo"] != 0.0))
    add_bin = bool(np.any(g["b_in"] != 0.0))
    add_bv = bool(np.any(bv != 0.0))
    key = (add_bo, add_bin, add_bv)
    if key not in _PROG_CACHE:
        _PROG_CACHE[key] = build_program(add_bo, add_bin=add_bin, add_bv=add_bv)
    nc = _PROG_CACHE[key]

    common = {
        "Win": np.ascontiguousarray(g["W_in"].astype(bf)),
        "Wq": np.ascontiguousarray(Wq.astype(bf)),
        "Wk": np.ascontiguousarray(Wk.astype(bf)),
        "Wv": np.ascontiguousarray(Wv.astype(bf)),
        "Wo": np.ascontiguousarray(g["Wo"].astype(bf)),
        "W1": np.ascontiguousarray(W1.astype(bf)),
        "W2": np.ascontiguousarray(g["W2"].astype(bf)),
    }
    in_maps = []
    for c in range(8):
        b, k = c // 2, c % 2
        im = dict(common)
        im["xmT"] = np.ascontiguousarray(x_m[b].T.astype(bf))
        sm = np.zeros((128, 36), f32)
        sm[:, 0:12] = A[b, :, k].reshape(12, 128).T
        sm[:, 12:16] = bq.reshape(4, 128).T
        sm[:, 16:20] = bk.reshape(4, 128).T
        sm[:, 20:36] = b1.reshape(16, 128).T
        im["smalls"] = sm
        rows = np.stack([g["b_in"], bv, g["b2"] + g["spk_tags"][k], g["bo"]])
        im["rows"] = rows.astype(f32)
        in_maps.append(im)

    res = run_bass_kernel_spmd(nc, in_maps, core_ids=list(range(8)))
    out = np.zeros((B, KSP * T, D), f32)
    gs, bs = g["ln_s_g"], g["ln_s_b"]
    for c in range(8):
        b, k = c // 2, c % 2
        # device output is (y3 - mean); finish LN_s here: divide by
        # sqrt(var+eps) (var shipped as [128, NT], col-major tiles), then
        # apply gain/bias.
        var = res.results[c]["var"]          # [128, NT]
        rstd = 1.0 / np.sqrt(var.T.reshape(T, 1) + 1e-5)
        out[b, k * T:(k + 1) * T] = res.results[c]["out"] * rstd * gs + bs
    return out



# revision 60
# speedup vs baseline: 1.0305x; 1.0305x over previous
"""Trainium2 Bass kernel for nn_CTCPerSpeakerExtractorConcatNNG.

Sharding: 8 cores = (batch b, speaker k) pairs; each core runs the full
T=1536 stream for its pair. No collectives; host scatters/gathers.

Per-core dataflow (natural layout [T-tiles x 128 part, D free], bf16 acts):
  X = xmT.T @ Win + bin               (xmT pre-transposed on host)
  LN_kv(X) -> transpose -> KVT -> KT (transposed), V (halo-tiled natural)
  Xk = X * sigmoid(6(A-.5));  LN_q -> transpose -> QT (transposed)
  banded attention (BAND=24) with 128-query tiles x 176-key windows
  y2 = Xk + attn@Wo ; LN_f -> transpose -> FFN (gelu) ; y3 = y2 + h2 + b2k
  out = LN_s(y3) normalized only; host applies ln_s gain/bias.
LN gains/biases for kv/q/f are folded into the following matmul on host.
"""
import sys

for _p in ("/opt/trn_rl_repo", "/root/.axon_site/_ro/trn_rl_repo"):
    if _p not in sys.path:
        sys.path.append(_p)

from contextlib import ExitStack

import numpy as np
import ml_dtypes

import concourse.bass as bass
import concourse.bacc as bacc
import concourse.tile as tile
from concourse import mybir
from concourse.bass_utils import run_bass_kernel_spmd
from concourse.masks import make_identity

BF = mybir.dt.bfloat16
F32 = mybir.dt.float32
I32 = mybir.dt.int32
AF = mybir.ActivationFunctionType
OP = mybir.AluOpType
MAGIC1 = 0x5F3759DF + 1

B, T, D, KSP, H, BAND = 4, 1536, 512, 2, 8, 24
DH = D // H          # 64
P = 128
NT = T // P          # 12
WIN = P + 2 * BAND   # 176
NC_D = D // P        # 4 chunks of contraction dim
DFF = 4 * D          # 2048
NDH = DFF // P       # 16
EPS = 1e-5

# V halo-tile starts (each tile = up to 128 rows starting at s)
_VSTARTS = sorted({0, 128, 1360, 1488} | {128 * m - 24 for m in range(1, 12)})
_VIDX = {s: j for j, s in enumerate(_VSTARTS)}
NV = len(_VSTARTS)   # 15


def _bcast_ap(dram_ap, parts=128):
    """[N] dram vector -> [parts, N] broadcast AP (partition step 0)."""
    return bass.AP(
        tensor=dram_ap.tensor,
        offset=dram_ap.offset,
        ap=[[0, parts]] + list(dram_ap.ap),
    )


def _ln_stats_into(nc, pool, in_ap, mv_out):
    """bn_stats/bn_aggr for one tile; (mean, var) land in mv_out [128, 2]."""
    st = pool.tile([P, 6], F32, tag="bn_st")
    nc.vector.bn_stats(out=st, in_=in_ap)
    nc.vector.bn_aggr(out=mv_out, in_=st)


def _rsqrt_cols(nc, pool, v_ap, out_ap, n):
    """out = (v + EPS)^-0.5 for [128, n] columns, DVE only (no act table).

    Quake-style seed: bits exact via shift+xor; the +MAGIC add runs through
    the DVE's fp32 ALU (rounds above 2^24), which only perturbs low mantissa
    seed bits. Two Newton iterations finish to ~5e-6 rel err."""
    vp = pool.tile([P, n], F32, tag="rs_vp")
    nc.vector.tensor_scalar_add(out=vp, in0=v_ap, scalar1=EPS)
    nh = pool.tile([P, n], I32, tag="rs_nh")
    nc.vector.tensor_scalar(out=nh, in0=vp[:, :].bitcast(I32), scalar1=1,
                            scalar2=-1, op0=OP.logical_shift_right,
                            op1=OP.bitwise_xor)
    y0i = pool.tile([P, n], I32, tag="rs_y0")
    nc.vector.tensor_scalar_add(out=y0i, in0=nh, scalar1=MAGIC1)
    y = y0i[:, :].bitcast(F32)
    for it in range(2):
        t1 = pool.tile([P, n], F32, tag=f"rs_t{it}")
        nc.vector.tensor_tensor(out=t1, in0=y, in1=y, op=OP.mult)
        nc.vector.tensor_tensor(out=t1, in0=t1, in1=vp, op=OP.mult)
        nc.vector.tensor_scalar(out=t1, in0=t1, scalar1=-0.5, scalar2=1.5,
                                op0=OP.mult, op1=OP.add)
        if it == 0:
            yn = pool.tile([P, n], F32, tag="rs_yn")
            nc.vector.tensor_tensor(out=yn, in0=t1, in1=y, op=OP.mult)
            y = yn
        else:
            nc.vector.tensor_tensor(out=out_ap, in0=t1, in1=y, op=OP.mult)


def build_program(add_bo: bool, stop_stage: int = 99, add_bin: bool = False,
                  add_bv: bool = False) -> bass.Bass:
    nc = bacc.Bacc()

    # ---- DRAM I/O ----
    xmT = nc.dram_tensor("xmT", [D, T], BF, kind="ExternalInput")
    Wd = {}
    for nm, (di, do) in [("Win", (D, D)), ("Wq", (D, D)), ("Wk", (D, D)),
                         ("Wv", (D, D)), ("Wo", (D, D)), ("W1", (D, DFF)),
                         ("W2", (DFF, D))]:
        Wd[nm] = nc.dram_tensor(nm, [di, do], BF, kind="ExternalInput")
    smalls_d = nc.dram_tensor("smalls", [P, 36], F32, kind="ExternalInput")
    rows_d = nc.dram_tensor("rows", [4, D], F32, kind="ExternalInput")
    out_d = nc.dram_tensor("out", [T, D], F32, kind="ExternalOutput")
    var_d = nc.dram_tensor("var", [P, NT], F32, kind="ExternalOutput")
    out_t = out_d.rearrange("(n p) d -> n p d", p=P)

    with tile.TileContext(nc) as tc, ExitStack() as ctx:
        def _body():
            consts = ctx.enter_context(tc.tile_pool(name="consts", bufs=1))
            wpool = ctx.enter_context(tc.tile_pool(name="wpool", bufs=1))
            acts = ctx.enter_context(tc.tile_pool(name="acts", bufs=1))
            ln_nat_p = ctx.enter_context(tc.tile_pool(name="ln_nat_p", bufs=2))
            tT_p = ctx.enter_context(tc.tile_pool(name="tT_p", bufs=1))
            kqh = ctx.enter_context(tc.tile_pool(name="kqh", bufs=1))
            kv_p = ctx.enter_context(tc.tile_pool(name="kv_p", bufs=1))
            small = ctx.enter_context(tc.tile_pool(name="small", bufs=6))
            sm2 = ctx.enter_context(tc.tile_pool(name="sm2", bufs=2))
            outp = ctx.enter_context(tc.tile_pool(name="outp", bufs=2))
            psA = ctx.enter_context(tc.tile_pool(name="psA", bufs=2, space="PSUM"))
            psT = ctx.enter_context(tc.tile_pool(name="psT", bufs=1, space="PSUM"))
            psB = ctx.enter_context(tc.tile_pool(name="psB", bufs=3, space="PSUM"))
            psD = ctx.enter_context(tc.tile_pool(name="psD", bufs=1, space="PSUM"))
            psE = ctx.enter_context(tc.tile_pool(name="psE", bufs=1, space="PSUM"))

            def dbg_out(get_tile_view, grouped=False):
                # copy 12 [128,512] views (cast to f32) to out and stop
                for _mt in range(NT):
                    o = outp.tile([P, D], F32, tag="o_sb")
                    ov = o.rearrange("p (c q) -> p c q", c=NC_D) if grouped else o
                    nc.vector.tensor_copy(out=ov, in_=get_tile_view(_mt))
                    nc.sync.dma_start(out=out_t[_mt], in_=o)

            # ---- constants ----
            ident = consts.tile([P, P], BF)
            make_identity(nc, ident)
            eps_t = consts.tile([P, 1], F32, tag="eps_t")
            nc.vector.memset(eps_t, EPS)
            pos3_t = consts.tile([P, 1], F32, tag="pos3_t")
            nc.vector.memset(pos3_t, 3.0)
            ones_c = consts.tile([P, 1], BF, tag="ones_c")
            nc.vector.memset(ones_c, 1.0)
            var_sb = consts.tile([P, NT], F32, tag="var_sb")
            # Transposed band masks: mk[j, q] (j = key index in window on the
            # partition axis, q = query on the free axis), in two chunks
            # (A: j 0..127, B: j 128..175 on partitions 0..47).
            # keep iff 0 <= (j - q - off) <= 2*BAND.
            # masks[nm] layout [128, 2, 128]: chunk A ([:, 0, :], key j =
            # partition) and chunk B for even heads ([0:48, 1, :], key j =
            # 128 + partition). masksB64[nm] [128, 128] holds the chunk-B
            # mask on partitions 64:112 (key j = 128 + p - 64) so the odd
            # heads' mask-add matmul can keep its lhsT at partition offset
            # 64 — an accumulation group whose matmuls mix lhsT partition
            # offsets (64 then 0) at M<128 wedges the PE at runtime.
            masks, masksB64 = {}, {}
            NEG = -1e30
            for off, nm in ((0, "mid"), (-BAND, "first"), (BAND, "last")):
                mk = consts.tile([P, 2, P], BF, tag=f"mask_{nm}")
                nc.gpsimd.memset(mk, 0.0)
                for ch, jo in ((0, 0), (1, P)):
                    v = mk[:, ch, :]
                    # keep iff (j - q - off) >= 0 else -inf
                    nc.gpsimd.affine_select(
                        out=v, in_=v, compare_op=OP.is_ge, fill=NEG,
                        base=jo - off, pattern=[[-1, P]], channel_multiplier=1)
                    # keep iff (q + off + 2*BAND - j) >= 0 else -inf
                    nc.gpsimd.affine_select(
                        out=v, in_=v, compare_op=OP.is_ge, fill=NEG,
                        base=off + 2 * BAND - jo, pattern=[[1, P]],
                        channel_multiplier=-1)
                masks[nm] = mk
                mb = consts.tile([P, P], BF, tag=f"maskB64_{nm}")
                nc.gpsimd.memset(mb, 0.0)
                # key j = 128 + p - 64: keep iff 0 <= (j - q - off) <= 2*BAND
                nc.gpsimd.affine_select(
                    out=mb, in_=mb, compare_op=OP.is_ge, fill=NEG,
                    base=64 - off, pattern=[[-1, P]], channel_multiplier=1)
                nc.gpsimd.affine_select(
                    out=mb, in_=mb, compare_op=OP.is_ge, fill=NEG,
                    base=off + 2 * BAND - 64, pattern=[[1, P]],
                    channel_multiplier=-1)
                masksB64[nm] = mb
            # identity block at partition offset 64: ident_s64[p, j] = 1 iff
            # p == j + 64 (rows 64:112 x cols 0:48 form I_48)
            ident_s64 = consts.tile([P, 48], BF, tag="ident_s64")
            nc.gpsimd.memset(ident_s64, 1.0)
            nc.gpsimd.affine_select(
                out=ident_s64, in_=ident_s64, compare_op=OP.is_equal, fill=0.0,
                base=-64, pattern=[[-1, 48]], channel_multiplier=1)

            def mask_name(mt):
                return "first" if mt == 0 else ("last" if mt == NT - 1 else "mid")

            def ws_of(mt):
                return min(max(mt * P - BAND, 0), T - WIN)

            # packed small tensors first: they gate What/Xk and the whole X
            # pipeline, and cost ~0.5us vs ~20us of weight DMA ahead of them.
            # cols [0:12]=A_k tiled, [12:16]=bq4, [16:20]=bk4, [20:36]=b1_16
            smalls = consts.tile([P, 36], F32, tag="smalls")
            nc.sync.dma_start(out=smalls, in_=smalls_d[:, :])

            # Win rides ahead of the big xmT transfer (both gate the first X
            # matmuls), and xmT lands in two column halves so X starts before
            # the whole activation tensor arrives.
            Ws = {}
            for nm, tag in (("Win", "WinWo"), ("Wk", "WkW1"), ("Wv", "Wv"),
                            ("Wq", "WqW2")):
                di, do = Wd[nm].shape
                w_t = wpool.tile([P, di // P, do], BF, tag=tag, name=f"w_{nm}")
                Ws[nm] = w_t
            nc.sync.dma_start(out=Ws["Win"],
                              in_=Wd["Win"].rearrange("(c p) o -> p c o", p=P))
            xmT_s = ln_nat_p.tile([P, NC_D, T], BF, tag="ln_nat")
            xmT_r = xmT.rearrange("(c p) t -> p c t", p=P)
            nc.sync.dma_start(out=xmT_s[:, :, 0:T // 2], in_=xmT_r[:, :, 0:T // 2])
            nc.sync.dma_start(out=xmT_s[:, :, T // 2:T], in_=xmT_r[:, :, T // 2:T])

            # ---- weights/biases to SBUF (order: earliest-needed first).
            # W1/W2 (4MB, needed only at the FFN) go through the Pool queue so
            # they stream in parallel with SP's projection weights.
            # Tag sharing reuses dead weight buffers: Wo lands in Win's spot
            # once the X stage drained it; W1/W2 land in Wk/Wq's (their DMAs
            # ride the idle Pool queue and block on the tile-free semaphore).
            for nm in ("Wk", "Wv", "Wq"):
                nc.sync.dma_start(out=Ws[nm],
                                  in_=Wd[nm].rearrange("(c p) o -> p c o", p=P))
            t = wpool.tile([P, NC_D, D], BF, tag="WinWo")
            nc.sync.dma_start(out=t, in_=Wd["Wo"].rearrange("(c p) o -> p c o", p=P))
            Ws["Wo"] = t
            t = wpool.tile([P, NC_D, DFF], BF, tag="WkW1")
            nc.gpsimd.dma_start(out=t, in_=Wd["W1"].rearrange("(c p) o -> p c o", p=P))
            Ws["W1"] = t
            W2s = wpool.tile([P, NDH, D], BF, tag="WqW2")
            nc.gpsimd.dma_start(out=W2s, in_=Wd["W2"].rearrange("(c p) o -> p c o", p=P))
            Ak_s = smalls[:, 0:NT]
            bq4 = smalls[:, 12:12 + NC_D]
            bk4 = smalls[:, 16:16 + NC_D]
            b1_16 = smalls[:, 20:20 + NDH]
            # bias row broadcasts (partition-step-0 DMA reads, contiguous source)
            if add_bin:
                bin_b = consts.tile([P, D], F32, tag="bin_b")
                nc.sync.dma_start(out=bin_b, in_=_bcast_ap(rows_d[0, :]))
            if add_bv:
                bv_b = consts.tile([P, D], F32, tag="bv_b")
                nc.sync.dma_start(out=bv_b, in_=_bcast_ap(rows_d[1, :]))
            b2k_b = consts.tile([P, D], F32, tag="b2k_b")
            nc.sync.dma_start(out=b2k_b, in_=_bcast_ap(rows_d[2, :]))
            if add_bo:
                ones_r = consts.tile([1, P], BF, tag="ones_r")
                nc.vector.memset(ones_r, 1.0)
                bo_rf = consts.tile([1, D], F32, tag="bo_rf")
                nc.sync.dma_start(out=bo_rf, in_=rows_d[3:4, :])
                bo_rb = consts.tile([1, D], BF, tag="bo_rb")
                nc.vector.tensor_copy(out=bo_rb, in_=bo_rf)

            # sigmoid(6(A-.5)) = 1/(1+exp(-6A+3)) — stays in the Exp act table
            What_e = consts.tile([P, NT], F32, tag="What_e")
            nc.scalar.activation(out=What_e, in_=Ak_s, func=AF.Exp, scale=-6.0,
                                 bias=pos3_t)
            nc.vector.tensor_scalar_add(out=What_e, in0=What_e, scalar1=1.0)
            What = consts.tile([P, NT], F32, tag="What")
            nc.vector.reciprocal(out=What, in_=What_e)

            # ---- X = xmT.T @ Win + bin ; LN_kv ; Xk (X stays in PSUM only) ----
            # Only Xk = X * What leaves the PSUM; LN_kv is reconstructed from
            # Xk as lnkv = Xk * (rstd/What) - mean*rstd (relative bf16 error
            # is unchanged by the gate divide). rstd comes from the DVE-only
            # Newton rsqrt, batched per 4-tile group so the Activation engine
            # runs no LN work (and no Sqrt/Ln act-table loads, ever).
            rWhat = consts.tile([P, NT], F32, tag="rWhat")
            nc.vector.reciprocal(out=rWhat, in_=What)
            mv_kv = consts.tile([P, 2, NT], F32, tag="mv_kv")
            rstd_kv = consts.tile([P, NT], F32, tag="rstd_kv")
            lnkv = ln_nat_p.tile([P, NT, D], BF, tag="ln_nat")
            Xk = acts.tile([P, NT, D], BF, tag="Xk")
            for g in range(NT // 4):
                for i in range(4):
                    mt = 4 * g + i
                    ps = psA.tile([P, D], F32, tag="psA")
                    for c in range(NC_D):
                        nc.tensor.matmul(
                            ps, lhsT=xmT_s[:, c, mt * P:(mt + 1) * P],
                            rhs=Ws["Win"][:, c, :],
                            start=(c == 0), stop=(c == NC_D - 1))
                    if add_bin:
                        psb = sm2.tile([P, D], F32, tag="Xpsb")
                        nc.vector.tensor_tensor(out=psb, in0=ps, in1=bin_b, op=OP.add)
                    else:
                        psb = ps
                    _ln_stats_into(nc, small, psb, mv_kv[:, :, mt])
                    nc.scalar.activation(out=Xk[:, mt, :], in_=psb, func=AF.Copy,
                                         scale=What[:, mt:mt + 1])
                _rsqrt_cols(nc, small, mv_kv[:, 1, 4 * g:4 * g + 4],
                            rstd_kv[:, 4 * g:4 * g + 4], 4)
                for i in range(4):
                    mt = 4 * g + i
                    s1 = small.tile([P, 1], F32, tag="ln_s1")
                    nc.vector.tensor_tensor(out=s1, in0=rstd_kv[:, mt:mt + 1],
                                            in1=rWhat[:, mt:mt + 1], op=OP.mult)
                    s2 = small.tile([P, 1], F32, tag="ln_s2")
                    nc.vector.tensor_tensor(out=s2, in0=mv_kv[:, 0, mt:mt + 1],
                                            in1=rstd_kv[:, mt:mt + 1], op=OP.mult)
                    nc.vector.tensor_scalar(out=lnkv[:, mt, :], in0=Xk[:, mt, :],
                                            scalar1=s1, scalar2=s2,
                                            op0=OP.mult, op1=OP.subtract)

            if stop_stage == 1:
                dbg_out(lambda m: lnkv[:, m, :])
                return

            # ---- transpose LN_kv -> KVT [128, 4, T] ----
            # The PSUM->SBUF copy after each transpose runs on the Activation
            # engine: the pre-attention region is DVE-bound while Act has
            # headroom.
            def transpose_nat_to_T(src, dst):
                for mt in range(NT):
                    pt = psT.tile([P, NC_D * P], BF, tag="psT")
                    for c in range(NC_D):
                        nc.tensor.transpose(
                            pt[:, c * P:(c + 1) * P], src[:, mt, c * P:(c + 1) * P], ident)
                    v = pt.rearrange("p (c q) -> p c q", c=NC_D)
                    if mt % 2 == 0:
                        nc.scalar.copy(out=dst[:, :, mt * P:(mt + 1) * P], in_=v)
                    else:
                        nc.vector.tensor_copy(
                            out=dst[:, :, mt * P:(mt + 1) * P], in_=v)

            KVT = tT_p.tile([P, NC_D, T], BF, tag="tT")
            transpose_nat_to_T(lnkv, KVT)

            if stop_stage == 2:
                dbg_out(lambda m: KVT[:, :, m * P:(m + 1) * P], grouped=True)
                return

            # ---- KT (transposed) and V (halo natural) ----
            KT = kv_p.tile([P, NC_D, T], BF, tag="KT")
            for co in range(NC_D):
                for tch in range(3):
                    ps = psA.tile([P, D], F32, tag="psA")
                    for c in range(NC_D):
                        nc.tensor.matmul(
                            ps, lhsT=Ws["Wk"][:, c, co * P:(co + 1) * P],
                            rhs=KVT[:, c, tch * D:(tch + 1) * D],
                            start=(c == 0), stop=(c == NC_D - 1))
                    nc.scalar.activation(
                        out=KT[:, co, tch * D:(tch + 1) * D], in_=ps,
                        func=AF.Identity, bias=bk4[:, co:co + 1])

            Vh = acts.tile([P, NV, D], BF, tag="Vh")
            for j, s in enumerate(_VSTARTS):
                w = min(P, T - s)
                ps = psA.tile([P, D], F32, tag="psA")
                for c in range(NC_D):
                    nc.tensor.matmul(
                        ps[0:w, :], lhsT=KVT[:, c, s:s + w], rhs=Ws["Wv"][:, c, :],
                        start=(c == 0), stop=(c == NC_D - 1))
                if add_bv:
                    nc.vector.tensor_tensor(
                        out=Vh[0:w, j, :], in0=ps[0:w, :], in1=bv_b[0:w, :], op=OP.add)
                else:
                    nc.vector.tensor_copy(out=Vh[0:w, j, :], in_=ps[0:w, :])

            if stop_stage == 3:
                dbg_out(lambda m: Vh[:, m, :])
                return

            # ---- LN_q on Xk ; transpose ; QT ----
            mv_q = consts.tile([P, 2, NT], F32, tag="mv_q")
            rstd_q = consts.tile([P, NT], F32, tag="rstd_q")
            lnq = ln_nat_p.tile([P, NT, D], BF, tag="ln_nat")
            for g in range(NT // 4):
                for i in range(4):
                    mt = 4 * g + i
                    _ln_stats_into(nc, small, Xk[:, mt, :], mv_q[:, :, mt])
                _rsqrt_cols(nc, small, mv_q[:, 1, 4 * g:4 * g + 4],
                            rstd_q[:, 4 * g:4 * g + 4], 4)
                for i in range(4):
                    mt = 4 * g + i
                    nc.vector.tensor_scalar(out=lnq[:, mt, :], in0=Xk[:, mt, :],
                                            scalar1=mv_q[:, 0, mt:mt + 1],
                                            scalar2=rstd_q[:, mt:mt + 1],
                                            op0=OP.subtract, op1=OP.mult)
            LNQT = tT_p.tile([P, NC_D, T], BF, tag="tT")
            transpose_nat_to_T(lnq, LNQT)

            QT = kv_p.tile([P, NC_D, T], BF, tag="QT")
            for co in range(NC_D):
                for tch in range(3):
                    ps = psA.tile([P, D], F32, tag="psA")
                    for c in range(NC_D):
                        nc.tensor.matmul(
                            ps, lhsT=Ws["Wq"][:, c, co * P:(co + 1) * P],
                            rhs=LNQT[:, c, tch * D:(tch + 1) * D],
                            start=(c == 0), stop=(c == NC_D - 1))
                    nc.scalar.activation(
                        out=QT[:, co, tch * D:(tch + 1) * D], in_=ps,
                        func=AF.Identity, bias=bq4[:, co:co + 1])

            if stop_stage == 4:
                dbg_out(lambda m: QT[:, :, m * P:(m + 1) * P], grouped=True)
                return

            # ---- attention (transposed scores) + inline Wo/residual/LN_f ----
            # Scores are computed transposed: sT[j, q] = k_j . q_q, two window
            # chunks (A: keys ws..ws+127, B: ws+128..ws+175 on partitions
            # 0..47). exp(sT) is then directly the lhsT of the AV matmul
            # (out natural [q, d]) — no probability transposes or PSUM->SBUF
            # prob copies. Denominators come from N=1 matmuls against a ones
            # column; normalization happens after AV with per-row scalars.
            # Wo + residual + LN_f stats run per tile right after the
            # transpose, so no full-T attention output is ever stored.
            mv_f = consts.tile([P, 2, NT], F32, tag="mv_f")
            rstd_f = consts.tile([P, NT], F32, tag="rstd_f")
            y2 = acts.tile([P, NT, D], BF, tag="y2")
            lnf = ln_nat_p.tile([P, NT, D], BF, tag="ln_nat")
            inv_sqrt_dh = 1.0 / float(np.sqrt(DH))
            attn_tiles = [0] if stop_stage in (41, 42, 43) else range(NT)
            for mt in attn_tiles:
                ws = ws_of(mt)
                mk = masks[mask_name(mt)]
                mkB64 = masksB64[mask_name(mt)]
                if mt == 0:
                    j1, j2 = _VIDX[0], _VIDX[128]
                elif mt == NT - 1:
                    j1, j2 = _VIDX[1360], _VIDX[1488]
                else:
                    j1, j2 = _VIDX[128 * mt - 24], _VIDX[128 * mt + 104]
                pT = sm2.tile([P, H, 2 * P], BF, tag="pT")
                for hpair in range(H // 2):
                    ps = psB.tile([P, 2, 2 * P], F32, tag="psB")
                    for hs in range(2):
                        h = 2 * hpair + hs
                        hp, hc = 64 * (h % 2), h // 2
                        # chunk A: 128 keys x 128 queries (mixed lhsT offsets
                        # are fine at M=128)
                        nc.tensor.matmul(
                            ps[:, hs, 0:P],
                            lhsT=KT[hp:hp + 64, hc, ws:ws + P],
                            rhs=QT[hp:hp + 64, hc, mt * P:(mt + 1) * P],
                            start=True, stop=False)
                        nc.tensor.matmul(ps[:, hs, 0:P], lhsT=ident,
                                         rhs=mk[:, 0, :], start=False, stop=True)
                        # chunk B: 48 keys x 128 queries. The mask-add lhsT
                        # must match the QK lhsT's partition offset (M<128
                        # groups with mixed offsets wedge the PE).
                        nc.tensor.matmul(
                            ps[0:48, hs, P:2 * P],
                            lhsT=KT[hp:hp + 64, hc, ws + P:ws + WIN],
                            rhs=QT[hp:hp + 64, hc, mt * P:(mt + 1) * P],
                            start=True, stop=False)
                        if hp == 0:
                            nc.tensor.matmul(
                                ps[0:48, hs, P:2 * P], lhsT=ident[0:48, 0:48],
                                rhs=mk[0:48, 1, :], start=False, stop=True)
                        else:
                            nc.tensor.matmul(
                                ps[0:48, hs, P:2 * P],
                                lhsT=ident_s64[64:112, :],
                                rhs=mkB64[64:112, :], start=False, stop=True)
                    # batched exp per chunk (partitions 48.. of the B chunks
                    # are uninitialized PSUM and are never read)
                    nc.scalar.activation(
                        out=pT[:, 2 * hpair:2 * hpair + 2, 0:P],
                        in_=ps[:, :, 0:P], func=AF.Exp, scale=inv_sqrt_dh)
                    nc.scalar.activation(
                        out=pT[0:48, 2 * hpair:2 * hpair + 2, P:2 * P],
                        in_=ps[0:48, :, P:2 * P], func=AF.Exp,
                        scale=inv_sqrt_dh)
                if stop_stage == 41:
                    dbg_out(lambda m: pT[:, 0:2, :].rearrange("p h w -> p (h w)"))
                    return
                psy = psD.tile([P, D], F32, tag="psD")
                psden = psE.tile([P, H], F32, tag="psE")
                for h in range(H):
                    nc.tensor.matmul(
                        psy[:, h * DH:(h + 1) * DH], lhsT=pT[:, h, 0:P],
                        rhs=Vh[:, j1, h * DH:(h + 1) * DH],
                        start=True, stop=False)
                    nc.tensor.matmul(
                        psy[:, h * DH:(h + 1) * DH], lhsT=pT[0:48, h, P:2 * P],
                        rhs=Vh[0:48, j2, h * DH:(h + 1) * DH],
                        start=False, stop=True)
                    nc.tensor.matmul(
                        psden[:, h:h + 1], lhsT=pT[:, h, 0:P],
                        rhs=ones_c[:, 0:1], start=True, stop=False)
                    nc.tensor.matmul(
                        psden[:, h:h + 1], lhsT=pT[0:48, h, P:2 * P],
                        rhs=ones_c[0:48, 0:1], start=False, stop=True)
                if stop_stage == 42:
                    dbg_out(lambda m: psy)
                    return
                rden = small.tile([P, H], F32, tag="rden")
                nc.vector.reciprocal(out=rden, in_=psden)
                y_nat = sm2.tile([P, D], BF, tag="y_nat")
                # normalize all 8 heads in one op: rden broadcast along each
                # head's 64 columns via a step-0 free-dim AP
                rap = rden[:, :]
                rb = bass.AP(tensor=rap.tensor, offset=rap.offset,
                             ap=[rap.ap[0], [rap.ap[-1][0], H], [0, DH]])
                nc.vector.tensor_tensor(
                    out=y_nat[:, :].rearrange("p (h w) -> p h w", h=H),
                    in0=psy[:, :].rearrange("p (h w) -> p h w", h=H),
                    in1=rb, op=OP.mult)
                if stop_stage == 43:
                    dbg_out(lambda m: y_nat)
                    return
                ptp = psT.tile([P, NC_D * P], BF, tag="psT")
                for c in range(NC_D):
                    nc.tensor.transpose(
                        ptp[:, c * P:(c + 1) * P], y_nat[:, c * P:(c + 1) * P],
                        ident)
                YTt = sm2.tile([P, NC_D * P], BF, tag="YTt")
                nc.vector.tensor_copy(out=YTt, in_=ptp)
                # Wo + residual for this tile
                ps = psA.tile([P, D], F32, tag="psA")
                for c in range(NC_D):
                    nc.tensor.matmul(
                        ps, lhsT=YTt[:, c * P:(c + 1) * P], rhs=Ws["Wo"][:, c, :],
                        start=(c == 0), stop=(c == NC_D - 1 and not add_bo))
                if add_bo:
                    nc.tensor.matmul(ps, lhsT=ones_r[:, 0:P], rhs=bo_rb,
                                     start=False, stop=True)
                nc.vector.tensor_tensor(
                    out=y2[:, mt, :], in0=ps, in1=Xk[:, mt, :], op=OP.add)
                _ln_stats_into(nc, small, y2[:, mt, :], mv_f[:, :, mt])
                if mt % 4 == 3:
                    g = mt // 4
                    _rsqrt_cols(nc, small, mv_f[:, 1, 4 * g:4 * g + 4],
                                rstd_f[:, 4 * g:4 * g + 4], 4)
                    for i in range(4):
                        m2 = 4 * g + i
                        nc.vector.tensor_scalar(
                            out=lnf[:, m2, :], in0=y2[:, m2, :],
                            scalar1=mv_f[:, 0, m2:m2 + 1],
                            scalar2=rstd_f[:, m2:m2 + 1],
                            op0=OP.subtract, op1=OP.mult)

            if stop_stage == 6:
                dbg_out(lambda m: lnf[:, m, :])
                return

            LNFT = tT_p.tile([P, NC_D, T], BF, tag="tT")
            transpose_nat_to_T(lnf, LNFT)

            # ---- FFN + final residual + LN_s + output ----
            for tch in range(3):
                H1g = kqh.tile([P, NDH, D], BF, tag="kqh")
                for dh in range(NDH):
                    ps = psA.tile([P, D], F32, tag="psA")
                    for c in range(NC_D):
                        nc.tensor.matmul(
                            ps, lhsT=Ws["W1"][:, c, dh * P:(dh + 1) * P],
                            rhs=LNFT[:, c, tch * D:(tch + 1) * D],
                            start=(c == 0), stop=(c == NC_D - 1))
                    nc.scalar.activation(out=H1g[:, dh, :], in_=ps, func=AF.Gelu,
                                         bias=b1_16[:, dh:dh + 1])
                for sub in range(4):
                    mtg = tch * 4 + sub
                    ps = psA.tile([P, D], F32, tag="psA")
                    for dh in range(NDH):
                        nc.tensor.matmul(
                            ps, lhsT=H1g[:, dh, sub * P:(sub + 1) * P],
                            rhs=W2s[:, dh, :], start=(dh == 0), stop=(dh == NDH - 1))
                    y3 = outp.tile([P, D], F32, tag="y3")
                    nc.vector.tensor_tensor(out=y3, in0=ps, in1=y2[:, mtg, :], op=OP.add)
                    y3b = y3
                    nc.gpsimd.tensor_tensor(out=y3b, in0=y3, in1=b2k_b, op=OP.add)
                    # final LN_s: only subtract the mean on device; the
                    # variance is shipped out and the host divides by
                    # sqrt(var+eps) (and applies gain/bias) — keeps Sqrt (a
                    # conflicting act table) out of the Gelu region.
                    st = small.tile([P, 6], F32)
                    nc.vector.bn_stats(out=st, in_=y3b)
                    mv = small.tile([P, 2], F32)
                    nc.vector.bn_aggr(out=mv, in_=st)
                    nc.vector.tensor_copy(out=var_sb[:, mtg:mtg + 1],
                                          in_=mv[:, 1:2])
                    o_sb = outp.tile([P, D], F32, tag="o_sb")
                    nc.vector.tensor_scalar_sub(out=o_sb, in0=y3b,
                                                scalar1=mv[:, 0:1])
                    nc.sync.dma_start(out=out_t[mtg], in_=o_sb)
            nc.sync.dma_start(out=var_d[:, :], in_=var_sb)

        _body()
    nc.finalize()
    return nc


_PROG_CACHE = {}


def kernel(**inputs) -> np.ndarray:
    f32 = np.float32
    bf = ml_dtypes.bfloat16
    x_m = np.asarray(inputs["x_m"], f32)
    A = np.asarray(inputs["A"], f32)
    g = {k: np.asarray(v, f32) for k, v in inputs.items()}

    # fold LN affine params into following matmuls (exact algebra)
    Wq = g["ln_q_g"][:, None] * g["Wq"]
    bq = g["bq"] + g["ln_q_b"] @ g["Wq"]
    Wk = g["ln_kv_g"][:, None] * g["Wk"]
    bk = g["bk"] + g["ln_kv_b"] @ g["Wk"]
    Wv = g["ln_kv_g"][:, None] * g["Wv"]
    bv = g["bv"] + g["ln_kv_b"] @ g["Wv"]
    W1 = g["ln_f_g"][:, None] * g["W1"]
    b1 = g["b1"] + g["ln_f_b"] @ g["W1"]

    add_bo = bool(np.any(g["bo"] != 0.0))
    add_bin = bool(np.any(g["b_in"] != 0.0))
    add_bv = bool(np.any(bv != 0.0))
    key = (add_bo, add_bin, add_bv)
    if key not in _PROG_CACHE:
        _PROG_CACHE[key] = build_program(add_bo, add_bin=add_bin, add_bv=add_bv)
    nc = _PROG_CACHE[key]

    common = {
        "Win": np.ascontiguousarray(g["W_in"].astype(bf)),
        "Wq": np.ascontiguousarray(Wq.astype(bf)),
        "Wk": np.ascontiguousarray(Wk.astype(bf)),
        "Wv": np.ascontiguousarray(Wv.astype(bf)),
        "Wo": np.ascontiguousarray(g["Wo"].astype(bf)),
        "W1": np.ascontiguousarray(W1.astype(bf)),
        "W2": np.ascontiguousarray(g["W2"].astype(bf)),
    }
    in_maps = []
    for c in range(8):
        b, k = c // 2, c % 2
        im = dict(common)
        im["xmT"] = np.ascontiguousarray(x_m[b].T.astype(bf))
        sm = np.zeros((128, 36), f32)
        sm[:, 0:12] = A[b, :, k].reshape(12, 128).T
        sm[:, 12:16] = bq.reshape(4, 128).T
        sm[:, 16:20] = bk.reshape(4, 128).T
        sm[:, 20:36] = b1.reshape(16, 128).T
        im["smalls"] = sm
        rows = np.stack([g["b_in"], bv, g["b2"] + g["spk_tags"][k], g["bo"]])
        im["rows"] = rows.astype(f32)
        in_maps.append(im)

    res = run_bass_kernel_spmd(nc, in_maps, core_ids=list(range(8)))
    out = np.zeros((B, KSP * T, D), f32)
    gs, bs = g["ln_s_g"], g["ln_s_b"]
    for c in range(8):
        b, k = c // 2, c % 2
        # device output is (y3 - mean); finish LN_s here: divide by
        # sqrt(var+eps) (var shipped as [128, NT], col-major tiles), then
        # apply gain/bias.
        var = res.results[c]["var"]          # [128, NT]
        rstd = 1.0 / np.sqrt(var.T.reshape(T, 1) + 1e-5)
        out[b, k * T:(k + 1) * T] = res.results[c]["out"] * rstd * gs + bs
    return out



# revision 67
# speedup vs baseline: 1.0671x; 1.0355x over previous
"""Trainium2 Bass kernel for nn_CTCPerSpeakerExtractorConcatNNG.

Sharding: 8 cores = (batch b, speaker k) pairs; each core runs the full
T=1536 stream for its pair. No collectives; host scatters/gathers.

Per-core dataflow (natural layout [T-tiles x 128 part, D free], bf16 acts):
  X = xmT.T @ Win + bin               (xmT pre-transposed on host)
  LN_kv(X) -> transpose -> KVT -> KT (transposed), V (halo-tiled natural)
  Xk = X * sigmoid(6(A-.5));  LN_q -> transpose -> QT (transposed)
  banded attention (BAND=24) with 128-query tiles x 176-key windows
  y2 = Xk + attn@Wo ; LN_f -> transpose -> FFN (gelu) ; y3 = y2 + h2 + b2k
  out = LN_s(y3) normalized only; host applies ln_s gain/bias.
LN gains/biases for kv/q/f are folded into the following matmul on host.
"""
import sys

for _p in ("/opt/trn_rl_repo", "/root/.axon_site/_ro/trn_rl_repo"):
    if _p not in sys.path:
        sys.path.append(_p)

from contextlib import ExitStack

import numpy as np
import ml_dtypes

import concourse.bass as bass
import concourse.bacc as bacc
import concourse.tile as tile
from concourse import mybir
from concourse.bass_utils import run_bass_kernel_spmd
from concourse.masks import make_identity

BF = mybir.dt.bfloat16
F32 = mybir.dt.float32
I32 = mybir.dt.int32
AF = mybir.ActivationFunctionType
OP = mybir.AluOpType
MAGIC1 = 0x5F3759DF + 1

B, T, D, KSP, H, BAND = 4, 1536, 512, 2, 8, 24
DH = D // H          # 64
P = 128
NT = T // P          # 12
WIN = P + 2 * BAND   # 176
NC_D = D // P        # 4 chunks of contraction dim
DFF = 4 * D          # 2048
NDH = DFF // P       # 16
EPS = 1e-5

# V halo-tile starts (each tile = up to 128 rows starting at s)
_VSTARTS = sorted({0, 128, 1360, 1488} | {128 * m - 24 for m in range(1, 12)})
_VIDX = {s: j for j, s in enumerate(_VSTARTS)}
NV = len(_VSTARTS)   # 15


def _bcast_ap(dram_ap, parts=128):
    """[N] dram vector -> [parts, N] broadcast AP (partition step 0)."""
    return bass.AP(
        tensor=dram_ap.tensor,
        offset=dram_ap.offset,
        ap=[[0, parts]] + list(dram_ap.ap),
    )


def _ln_stats_into(nc, pool, in_ap, mv_out):
    """bn_stats/bn_aggr for one tile; (mean, var) land in mv_out [128, 2]."""
    st = pool.tile([P, 6], F32, tag="bn_st")
    nc.vector.bn_stats(out=st, in_=in_ap)
    nc.vector.bn_aggr(out=mv_out, in_=st)


def _rsqrt_cols(nc, pool, v_ap, out_ap, n, eng=None):
    """out = (v + EPS)^-0.5 for [128, n] columns, no act table needed.

    DVE only — walrus cannot lower the bitwise/int tensor_scalar ops on the
    GPSIMD/Pool engine. Quake-style seed: bits exact via shift+xor; the
    +MAGIC add runs through the fp32 ALU (rounds above 2^24), which only
    perturbs low mantissa seed bits. Two Newton iterations finish to ~5e-6
    rel err."""
    if eng is None:
        eng = nc.vector
    vp = pool.tile([P, n], F32, tag="rs_vp")
    eng.tensor_scalar_add(out=vp, in0=v_ap, scalar1=EPS)
    nh = pool.tile([P, n], I32, tag="rs_nh")
    eng.tensor_scalar(out=nh, in0=vp[:, :].bitcast(I32), scalar1=1,
                      scalar2=-1, op0=OP.logical_shift_right,
                      op1=OP.bitwise_xor)
    y0i = pool.tile([P, n], I32, tag="rs_y0")
    eng.tensor_scalar_add(out=y0i, in0=nh, scalar1=MAGIC1)
    y = y0i[:, :].bitcast(F32)
    for it in range(2):
        t1 = pool.tile([P, n], F32, tag=f"rs_t{it}")
        eng.tensor_tensor(out=t1, in0=y, in1=y, op=OP.mult)
        eng.tensor_tensor(out=t1, in0=t1, in1=vp, op=OP.mult)
        eng.tensor_scalar(out=t1, in0=t1, scalar1=-0.5, scalar2=1.5,
                          op0=OP.mult, op1=OP.add)
        if it == 0:
            yn = pool.tile([P, n], F32, tag="rs_yn")
            eng.tensor_tensor(out=yn, in0=t1, in1=y, op=OP.mult)
            y = yn
        else:
            eng.tensor_tensor(out=out_ap, in0=t1, in1=y, op=OP.mult)


def build_program(add_bo: bool, stop_stage: int = 99, add_bin: bool = False,
                  add_bv: bool = False) -> bass.Bass:
    nc = bacc.Bacc()

    # ---- DRAM I/O ----
    xmT = nc.dram_tensor("xmT", [D, T], BF, kind="ExternalInput")
    Wd = {}
    for nm, (di, do) in [("Win", (D, D)), ("Wq", (D, D)), ("Wk", (D, D)),
                         ("Wv", (D, D)), ("Wo", (D, D)), ("W1", (D, DFF)),
                         ("W2", (DFF, D))]:
        Wd[nm] = nc.dram_tensor(nm, [di, do], BF, kind="ExternalInput")
    smalls_d = nc.dram_tensor("smalls", [P, 36], F32, kind="ExternalInput")
    rows_d = nc.dram_tensor("rows", [4, D], F32, kind="ExternalInput")
    out_d = nc.dram_tensor("out", [T, D], F32, kind="ExternalOutput")
    var_d = nc.dram_tensor("var", [P, NT], F32, kind="ExternalOutput")
    out_t = out_d.rearrange("(n p) d -> n p d", p=P)

    with tile.TileContext(nc) as tc, ExitStack() as ctx:
        def _body():
            consts = ctx.enter_context(tc.tile_pool(name="consts", bufs=1))
            wpool = ctx.enter_context(tc.tile_pool(name="wpool", bufs=1))
            acts = ctx.enter_context(tc.tile_pool(name="acts", bufs=1))
            ln_nat_p = ctx.enter_context(tc.tile_pool(name="ln_nat_p", bufs=2))
            tT_p = ctx.enter_context(tc.tile_pool(name="tT_p", bufs=1))
            kqh = ctx.enter_context(tc.tile_pool(name="kqh", bufs=1))
            kv_p = ctx.enter_context(tc.tile_pool(name="kv_p", bufs=1))
            small = ctx.enter_context(tc.tile_pool(name="small", bufs=6))
            sm2 = ctx.enter_context(tc.tile_pool(name="sm2", bufs=2))
            outp = ctx.enter_context(tc.tile_pool(name="outp", bufs=2))
            psA = ctx.enter_context(tc.tile_pool(name="psA", bufs=2, space="PSUM"))
            psT = ctx.enter_context(tc.tile_pool(name="psT", bufs=2, space="PSUM"))
            psB = ctx.enter_context(tc.tile_pool(name="psB", bufs=2, space="PSUM"))
            psD = ctx.enter_context(tc.tile_pool(name="psD", bufs=1, space="PSUM"))
            psE = ctx.enter_context(tc.tile_pool(name="psE", bufs=1, space="PSUM"))

            def dbg_out(get_tile_view, grouped=False):
                # copy 12 [128,512] views (cast to f32) to out and stop
                for _mt in range(NT):
                    o = outp.tile([P, D], F32, tag="o_sb")
                    ov = o.rearrange("p (c q) -> p c q", c=NC_D) if grouped else o
                    nc.vector.tensor_copy(out=ov, in_=get_tile_view(_mt))
                    nc.sync.dma_start(out=out_t[_mt], in_=o)

            # ---- constants ----
            ident = consts.tile([P, P], BF)
            make_identity(nc, ident)
            eps_t = consts.tile([P, 1], F32, tag="eps_t")
            nc.vector.memset(eps_t, EPS)
            pos3_t = consts.tile([P, 1], F32, tag="pos3_t")
            nc.vector.memset(pos3_t, 3.0)
            ones_c = consts.tile([P, 1], BF, tag="ones_c")
            nc.vector.memset(ones_c, 1.0)
            var_sb = consts.tile([P, NT], F32, tag="var_sb")
            # Transposed band masks: mk[j, q] (j = key index in window on the
            # partition axis, q = query on the free axis), in two chunks
            # (A: j 0..127, B: j 128..175 on partitions 0..47).
            # keep iff 0 <= (j - q - off) <= 2*BAND.
            # masks[nm] layout [128, 2, 128]: chunk A ([:, 0, :], key j =
            # partition) and chunk B for even heads ([0:48, 1, :], key j =
            # 128 + partition). masksB64[nm] [128, 128] holds the chunk-B
            # mask on partitions 64:112 (key j = 128 + p - 64) so the odd
            # heads' mask-add matmul can keep its lhsT at partition offset
            # 64 — an accumulation group whose matmuls mix lhsT partition
            # offsets (64 then 0) at M<128 wedges the PE at runtime.
            masks, masksB64 = {}, {}
            NEG = -1e30
            for off, nm in ((0, "mid"), (-BAND, "first"), (BAND, "last")):
                mk = consts.tile([P, 2, P], BF, tag=f"mask_{nm}")
                nc.gpsimd.memset(mk, 0.0)
                for ch, jo in ((0, 0), (1, P)):
                    v = mk[:, ch, :]
                    # keep iff (j - q - off) >= 0 else -inf
                    nc.gpsimd.affine_select(
                        out=v, in_=v, compare_op=OP.is_ge, fill=NEG,
                        base=jo - off, pattern=[[-1, P]], channel_multiplier=1)
                    # keep iff (q + off + 2*BAND - j) >= 0 else -inf
                    nc.gpsimd.affine_select(
                        out=v, in_=v, compare_op=OP.is_ge, fill=NEG,
                        base=off + 2 * BAND - jo, pattern=[[1, P]],
                        channel_multiplier=-1)
                masks[nm] = mk
                mb = consts.tile([P, P], BF, tag=f"maskB64_{nm}")
                nc.gpsimd.memset(mb, 0.0)
                # key j = 128 + p - 64: keep iff 0 <= (j - q - off) <= 2*BAND
                nc.gpsimd.affine_select(
                    out=mb, in_=mb, compare_op=OP.is_ge, fill=NEG,
                    base=64 - off, pattern=[[-1, P]], channel_multiplier=1)
                nc.gpsimd.affine_select(
                    out=mb, in_=mb, compare_op=OP.is_ge, fill=NEG,
                    base=off + 2 * BAND - 64, pattern=[[1, P]],
                    channel_multiplier=-1)
                masksB64[nm] = mb
            # identity block at partition offset 64: ident_s64[p, j] = 1 iff
            # p == j + 64 (rows 64:112 x cols 0:48 form I_48)
            ident_s64 = consts.tile([P, 48], BF, tag="ident_s64")
            nc.gpsimd.memset(ident_s64, 1.0)
            nc.gpsimd.affine_select(
                out=ident_s64, in_=ident_s64, compare_op=OP.is_equal, fill=0.0,
                base=-64, pattern=[[-1, 48]], channel_multiplier=1)

            def mask_name(mt):
                return "first" if mt == 0 else ("last" if mt == NT - 1 else "mid")

            def ws_of(mt):
                return min(max(mt * P - BAND, 0), T - WIN)

            # packed small tensors first: they gate What/Xk and the whole X
            # pipeline, and cost ~0.5us vs ~20us of weight DMA ahead of them.
            # cols [0:12]=A_k tiled, [12:16]=bq4, [16:20]=bk4, [20:36]=b1_16
            smalls = consts.tile([P, 36], F32, tag="smalls")
            nc.sync.dma_start(out=smalls, in_=smalls_d[:, :])

            # Win rides ahead of the big xmT transfer (both gate the first X
            # matmuls), and xmT lands in two column halves so X starts before
            # the whole activation tensor arrives.
            Ws = {}
            for nm, tag in (("Win", "WinWo"), ("Wk", "WkW1"), ("Wv", "Wv"),
                            ("Wq", "WqW2")):
                di, do = Wd[nm].shape
                w_t = wpool.tile([P, di // P, do], BF, tag=tag, name=f"w_{nm}")
                Ws[nm] = w_t
            nc.sync.dma_start(out=Ws["Win"],
                              in_=Wd["Win"].rearrange("(c p) o -> p c o", p=P))
            xmT_s = ln_nat_p.tile([P, NC_D, T], BF, tag="ln_nat")
            xmT_r = xmT.rearrange("(c p) t -> p c t", p=P)
            nc.sync.dma_start(out=xmT_s[:, :, 0:T // 2], in_=xmT_r[:, :, 0:T // 2])
            nc.sync.dma_start(out=xmT_s[:, :, T // 2:T], in_=xmT_r[:, :, T // 2:T])

            # ---- weights/biases to SBUF (order: earliest-needed first).
            # W1/W2 (4MB, needed only at the FFN) go through the Pool queue so
            # they stream in parallel with SP's projection weights.
            # Tag sharing reuses dead weight buffers: Wo lands in Win's spot
            # once the X stage drained it; W1/W2 land in Wk/Wq's (their DMAs
            # ride the idle Pool queue and block on the tile-free semaphore).
            for nm in ("Wk", "Wv", "Wq"):
                nc.sync.dma_start(out=Ws[nm],
                                  in_=Wd[nm].rearrange("(c p) o -> p c o", p=P))
            t = wpool.tile([P, NC_D, D], BF, tag="WinWo")
            nc.sync.dma_start(out=t, in_=Wd["Wo"].rearrange("(c p) o -> p c o", p=P))
            Ws["Wo"] = t
            t = wpool.tile([P, NC_D, DFF], BF, tag="WkW1")
            nc.gpsimd.dma_start(out=t, in_=Wd["W1"].rearrange("(c p) o -> p c o", p=P))
            Ws["W1"] = t
            W2s = wpool.tile([P, NDH, D], BF, tag="WqW2")
            nc.gpsimd.dma_start(out=W2s, in_=Wd["W2"].rearrange("(c p) o -> p c o", p=P))
            Ak_s = smalls[:, 0:NT]
            bq4 = smalls[:, 12:12 + NC_D]
            bk4 = smalls[:, 16:16 + NC_D]
            b1_16 = smalls[:, 20:20 + NDH]
            # bias row broadcasts (partition-step-0 DMA reads, contiguous source)
            if add_bin:
                bin_b = consts.tile([P, D], F32, tag="bin_b")
                nc.sync.dma_start(out=bin_b, in_=_bcast_ap(rows_d[0, :]))
            if add_bv:
                bv_b = consts.tile([P, D], F32, tag="bv_b")
                nc.sync.dma_start(out=bv_b, in_=_bcast_ap(rows_d[1, :]))
            b2k_b = consts.tile([P, D], F32, tag="b2k_b")
            nc.sync.dma_start(out=b2k_b, in_=_bcast_ap(rows_d[2, :]))
            if add_bo:
                ones_r = consts.tile([1, P], BF, tag="ones_r")
                nc.vector.memset(ones_r, 1.0)
                bo_rf = consts.tile([1, D], F32, tag="bo_rf")
                nc.sync.dma_start(out=bo_rf, in_=rows_d[3:4, :])
                bo_rb = consts.tile([1, D], BF, tag="bo_rb")
                nc.vector.tensor_copy(out=bo_rb, in_=bo_rf)

            # sigmoid(6(A-.5)) = 1/(1+exp(-6A+3)) — stays in the Exp act table
            What_e = consts.tile([P, NT], F32, tag="What_e")
            nc.scalar.activation(out=What_e, in_=Ak_s, func=AF.Exp, scale=-6.0,
                                 bias=pos3_t)
            nc.vector.tensor_scalar_add(out=What_e, in0=What_e, scalar1=1.0)
            What = consts.tile([P, NT], F32, tag="What")
            nc.vector.reciprocal(out=What, in_=What_e)

            # ---- X = xmT.T @ Win + bin ; LN_kv ; Xk (X stays in PSUM only) ----
            # Only Xk = X * What leaves the PSUM; LN_kv is reconstructed from
            # Xk as lnkv = Xk * (rstd/What) - mean*rstd (relative bf16 error
            # is unchanged by the gate divide). rstd comes from the DVE-only
            # Newton rsqrt, batched per 4-tile group so the Activation engine
            # runs no LN work (and no Sqrt/Ln act-table loads, ever).
            rWhat = consts.tile([P, NT], F32, tag="rWhat")
            nc.vector.reciprocal(out=rWhat, in_=What)
            mv_kv = consts.tile([P, 2, NT], F32, tag="mv_kv")
            rstd_kv = consts.tile([P, NT], F32, tag="rstd_kv")
            lnkv = ln_nat_p.tile([P, NT, D], BF, tag="ln_nat")
            Xk = acts.tile([P, NT, D], BF, tag="Xk")
            for g in range(NT // 4):
                for i in range(4):
                    mt = 4 * g + i
                    ps = psA.tile([P, D], F32, tag="psA")
                    for c in range(NC_D):
                        nc.tensor.matmul(
                            ps, lhsT=xmT_s[:, c, mt * P:(mt + 1) * P],
                            rhs=Ws["Win"][:, c, :],
                            start=(c == 0), stop=(c == NC_D - 1))
                    if add_bin:
                        psb = sm2.tile([P, D], F32, tag="Xpsb")
                        nc.vector.tensor_tensor(out=psb, in0=ps, in1=bin_b, op=OP.add)
                    else:
                        psb = ps
                    _ln_stats_into(nc, small, psb, mv_kv[:, :, mt])
                    nc.scalar.activation(out=Xk[:, mt, :], in_=psb, func=AF.Copy,
                                         scale=What[:, mt:mt + 1])
                _rsqrt_cols(nc, small, mv_kv[:, 1, 4 * g:4 * g + 4],
                            rstd_kv[:, 4 * g:4 * g + 4], 4)
                for i in range(4):
                    mt = 4 * g + i
                    s1 = small.tile([P, 1], F32, tag="ln_s1")
                    nc.vector.tensor_tensor(out=s1, in0=rstd_kv[:, mt:mt + 1],
                                            in1=rWhat[:, mt:mt + 1], op=OP.mult)
                    s2 = small.tile([P, 1], F32, tag="ln_s2")
                    nc.vector.tensor_tensor(out=s2, in0=mv_kv[:, 0, mt:mt + 1],
                                            in1=rstd_kv[:, mt:mt + 1], op=OP.mult)
                    nc.vector.tensor_scalar(out=lnkv[:, mt, :], in0=Xk[:, mt, :],
                                            scalar1=s1, scalar2=s2,
                                            op0=OP.mult, op1=OP.subtract)

            if stop_stage == 1:
                dbg_out(lambda m: lnkv[:, m, :])
                return

            # ---- transpose LN_kv -> KVT [128, 4, T] ----
            # The PSUM->SBUF copy after each transpose runs on the Activation
            # engine: the pre-attention region is DVE-bound while Act has
            # headroom.
            def transpose_nat_to_T(src, dst):
                for mt in range(NT):
                    pt = psT.tile([P, NC_D * P], BF, tag="psT")
                    for c in range(NC_D):
                        nc.tensor.transpose(
                            pt[:, c * P:(c + 1) * P], src[:, mt, c * P:(c + 1) * P], ident)
                    v = pt.rearrange("p (c q) -> p c q", c=NC_D)
                    if mt % 2 == 0:
                        nc.scalar.copy(out=dst[:, :, mt * P:(mt + 1) * P], in_=v)
                    else:
                        nc.vector.tensor_copy(
                            out=dst[:, :, mt * P:(mt + 1) * P], in_=v)

            KVT = tT_p.tile([P, NC_D, T], BF, tag="tT")
            transpose_nat_to_T(lnkv, KVT)

            if stop_stage == 2:
                dbg_out(lambda m: KVT[:, :, m * P:(m + 1) * P], grouped=True)
                return

            # ---- KT (transposed) and V (halo natural) ----
            KT = kv_p.tile([P, NC_D, T], BF, tag="KT")
            for tch in range(3):
                for co in range(NC_D):
                    ps = psA.tile([P, D], F32, tag="psA")
                    for c in range(NC_D):
                        nc.tensor.matmul(
                            ps, lhsT=Ws["Wk"][:, c, co * P:(co + 1) * P],
                            rhs=KVT[:, c, tch * D:(tch + 1) * D],
                            start=(c == 0), stop=(c == NC_D - 1))
                    nc.scalar.activation(
                        out=KT[:, co, tch * D:(tch + 1) * D], in_=ps,
                        func=AF.Identity, bias=bk4[:, co:co + 1])

            Vh = acts.tile([P, NV, D], BF, tag="Vh")
            for j, s in enumerate(_VSTARTS):
                w = min(P, T - s)
                ps = psA.tile([P, D], F32, tag="psA")
                for c in range(NC_D):
                    nc.tensor.matmul(
                        ps[0:w, :], lhsT=KVT[:, c, s:s + w], rhs=Ws["Wv"][:, c, :],
                        start=(c == 0), stop=(c == NC_D - 1))
                if add_bv:
                    nc.vector.tensor_tensor(
                        out=Vh[0:w, j, :], in0=ps[0:w, :], in1=bv_b[0:w, :], op=OP.add)
                else:
                    nc.vector.tensor_copy(out=Vh[0:w, j, :], in_=ps[0:w, :])

            if stop_stage == 3:
                dbg_out(lambda m: Vh[:, m, :])
                return

            # ---- LN_q on Xk ; transpose ; QT ----
            mv_q = consts.tile([P, 2, NT], F32, tag="mv_q")
            rstd_q = consts.tile([P, NT], F32, tag="rstd_q")
            lnq = ln_nat_p.tile([P, NT, D], BF, tag="ln_nat")
            for g in range(NT // 4):
                for i in range(4):
                    mt = 4 * g + i
                    _ln_stats_into(nc, small, Xk[:, mt, :], mv_q[:, :, mt])
                _rsqrt_cols(nc, small, mv_q[:, 1, 4 * g:4 * g + 4],
                            rstd_q[:, 4 * g:4 * g + 4], 4)
                for i in range(4):
                    mt = 4 * g + i
                    nc.vector.tensor_scalar(out=lnq[:, mt, :], in0=Xk[:, mt, :],
                                            scalar1=mv_q[:, 0, mt:mt + 1],
                                            scalar2=rstd_q[:, mt:mt + 1],
                                            op0=OP.subtract, op1=OP.mult)
            LNQT = tT_p.tile([P, NC_D, T], BF, tag="tT")
            transpose_nat_to_T(lnq, LNQT)

            QT = kv_p.tile([P, NC_D, T], BF, tag="QT")
            for tch in range(3):
                for co in range(NC_D):
                    ps = psA.tile([P, D], F32, tag="psA")
                    for c in range(NC_D):
                        nc.tensor.matmul(
                            ps, lhsT=Ws["Wq"][:, c, co * P:(co + 1) * P],
                            rhs=LNQT[:, c, tch * D:(tch + 1) * D],
                            start=(c == 0), stop=(c == NC_D - 1))
                    nc.scalar.activation(
                        out=QT[:, co, tch * D:(tch + 1) * D], in_=ps,
                        func=AF.Identity, bias=bq4[:, co:co + 1])

            if stop_stage == 4:
                dbg_out(lambda m: QT[:, :, m * P:(m + 1) * P], grouped=True)
                return

            # ---- attention (transposed scores) + inline Wo/residual/LN_f ----
            # Scores are computed transposed: sT[j, q] = k_j . q_q, two window
            # chunks (A: keys ws..ws+127, B: ws+128..ws+175 on partitions
            # 0..47). exp(sT) is then directly the lhsT of the AV matmul
            # (out natural [q, d]) — no probability transposes or PSUM->SBUF
            # prob copies. Denominators come from N=1 matmuls against a ones
            # column; normalization happens after AV with per-row scalars.
            # Wo + residual + LN_f stats run per tile right after the
            # transpose, so no full-T attention output is ever stored.
            mv_f = consts.tile([P, 2, NT], F32, tag="mv_f")
            rstd_f = consts.tile([P, NT], F32, tag="rstd_f")
            y2 = acts.tile([P, NT, D], BF, tag="y2")
            lnf = ln_nat_p.tile([P, NT, D], BF, tag="ln_nat")
            inv_sqrt_dh = 1.0 / float(np.sqrt(DH))
            attn_tiles = [0] if stop_stage in (41, 42, 43) else range(NT)
            for mt in attn_tiles:
                ws = ws_of(mt)
                mk = masks[mask_name(mt)]
                mkB64 = masksB64[mask_name(mt)]
                if mt == 0:
                    j1, j2 = _VIDX[0], _VIDX[128]
                elif mt == NT - 1:
                    j1, j2 = _VIDX[1360], _VIDX[1488]
                else:
                    j1, j2 = _VIDX[128 * mt - 24], _VIDX[128 * mt + 104]
                pT = sm2.tile([P, H, 2 * P], BF, tag="pT")
                for hpair in range(H // 2):
                    ps = psB.tile([P, 2, 2 * P], F32, tag="psB")
                    for hs in range(2):
                        h = 2 * hpair + hs
                        hp, hc = 64 * (h % 2), h // 2
                        # chunk A: 128 keys x 128 queries (mixed lhsT offsets
                        # are fine at M=128)
                        nc.tensor.matmul(
                            ps[:, hs, 0:P],
                            lhsT=KT[hp:hp + 64, hc, ws:ws + P],
                            rhs=QT[hp:hp + 64, hc, mt * P:(mt + 1) * P],
                            start=True, stop=False)
                        nc.tensor.matmul(ps[:, hs, 0:P], lhsT=ident,
                                         rhs=mk[:, 0, :], start=False, stop=True)
                        # chunk B: 48 keys x 128 queries. The mask-add lhsT
                        # must match the QK lhsT's partition offset (M<128
                        # groups with mixed offsets wedge the PE).
                        nc.tensor.matmul(
                            ps[0:48, hs, P:2 * P],
                            lhsT=KT[hp:hp + 64, hc, ws + P:ws + WIN],
                            rhs=QT[hp:hp + 64, hc, mt * P:(mt + 1) * P],
                            start=True, stop=False)
                        if hp == 0:
                            nc.tensor.matmul(
                                ps[0:48, hs, P:2 * P], lhsT=ident[0:48, 0:48],
                                rhs=mk[0:48, 1, :], start=False, stop=True)
                        else:
                            nc.tensor.matmul(
                                ps[0:48, hs, P:2 * P],
                                lhsT=ident_s64[64:112, :],
                                rhs=mkB64[64:112, :], start=False, stop=True)
                    # batched exp per chunk (partitions 48.. of the B chunks
                    # are uninitialized PSUM and are never read)
                    nc.scalar.activation(
                        out=pT[:, 2 * hpair:2 * hpair + 2, 0:P],
                        in_=ps[:, :, 0:P], func=AF.Exp, scale=inv_sqrt_dh)
                    nc.scalar.activation(
                        out=pT[0:48, 2 * hpair:2 * hpair + 2, P:2 * P],
                        in_=ps[0:48, :, P:2 * P], func=AF.Exp,
                        scale=inv_sqrt_dh)
                if stop_stage == 41:
                    dbg_out(lambda m: pT[:, 0:2, :].rearrange("p h w -> p (h w)"))
                    return
                psy = psD.tile([P, D], F32, tag="psD")
                psden = psE.tile([P, H], F32, tag="psE")
                for h in range(H):
                    nc.tensor.matmul(
                        psy[:, h * DH:(h + 1) * DH], lhsT=pT[:, h, 0:P],
                        rhs=Vh[:, j1, h * DH:(h + 1) * DH],
                        start=True, stop=False)
                    nc.tensor.matmul(
                        psy[:, h * DH:(h + 1) * DH], lhsT=pT[0:48, h, P:2 * P],
                        rhs=Vh[0:48, j2, h * DH:(h + 1) * DH],
                        start=False, stop=True)
                    nc.tensor.matmul(
                        psden[:, h:h + 1], lhsT=pT[:, h, 0:P],
                        rhs=ones_c[:, 0:1], start=True, stop=False)
                    nc.tensor.matmul(
                        psden[:, h:h + 1], lhsT=pT[0:48, h, P:2 * P],
                        rhs=ones_c[0:48, 0:1], start=False, stop=True)
                if stop_stage == 42:
                    dbg_out(lambda m: psy)
                    return
                rden = small.tile([P, H], F32, tag="rden")
                nc.vector.reciprocal(out=rden, in_=psden)
                y_nat = sm2.tile([P, D], BF, tag="y_nat")
                # normalize all 8 heads in one op: rden broadcast along each
                # head's 64 columns via a step-0 free-dim AP
                rap = rden[:, :]
                rb = bass.AP(tensor=rap.tensor, offset=rap.offset,
                             ap=[rap.ap[0], [rap.ap[-1][0], H], [0, DH]])
                nc.vector.tensor_tensor(
                    out=y_nat[:, :].rearrange("p (h w) -> p h w", h=H),
                    in0=psy[:, :].rearrange("p (h w) -> p h w", h=H),
                    in1=rb, op=OP.mult)
                if stop_stage == 43:
                    dbg_out(lambda m: y_nat)
                    return
                ptp = psT.tile([P, NC_D * P], BF, tag="psT")
                for c in range(NC_D):
                    nc.tensor.transpose(
                        ptp[:, c * P:(c + 1) * P], y_nat[:, c * P:(c + 1) * P],
                        ident)
                YTt = sm2.tile([P, NC_D * P], BF, tag="YTt")
                nc.vector.tensor_copy(out=YTt, in_=ptp)
                # Wo + residual for this tile
                ps = psA.tile([P, D], F32, tag="psA")
                for c in range(NC_D):
                    nc.tensor.matmul(
                        ps, lhsT=YTt[:, c * P:(c + 1) * P], rhs=Ws["Wo"][:, c, :],
                        start=(c == 0), stop=(c == NC_D - 1 and not add_bo))
                if add_bo:
                    nc.tensor.matmul(ps, lhsT=ones_r[:, 0:P], rhs=bo_rb,
                                     start=False, stop=True)
                nc.vector.tensor_tensor(
                    out=y2[:, mt, :], in0=ps, in1=Xk[:, mt, :], op=OP.add)
                _ln_stats_into(nc, small, y2[:, mt, :], mv_f[:, :, mt])
                if mt % 4 == 3:
                    g = mt // 4
                    _rsqrt_cols(nc, small, mv_f[:, 1, 4 * g:4 * g + 4],
                                rstd_f[:, 4 * g:4 * g + 4], 4)
                    for i in range(4):
                        m2 = 4 * g + i
                        nc.vector.tensor_scalar(
                            out=lnf[:, m2, :], in0=y2[:, m2, :],
                            scalar1=mv_f[:, 0, m2:m2 + 1],
                            scalar2=rstd_f[:, m2:m2 + 1],
                            op0=OP.subtract, op1=OP.mult)

            if stop_stage == 6:
                dbg_out(lambda m: lnf[:, m, :])
                return

            LNFT = tT_p.tile([P, NC_D, T], BF, tag="tT")
            transpose_nat_to_T(lnf, LNFT)

            # ---- FFN + final residual + LN_s + output ----
            for tch in range(3):
                H1g = kqh.tile([P, NDH, D], BF, tag="kqh")
                for dh in range(NDH):
                    ps = psA.tile([P, D], F32, tag="psA")
                    for c in range(NC_D):
                        nc.tensor.matmul(
                            ps, lhsT=Ws["W1"][:, c, dh * P:(dh + 1) * P],
                            rhs=LNFT[:, c, tch * D:(tch + 1) * D],
                            start=(c == 0), stop=(c == NC_D - 1))
                    nc.scalar.activation(out=H1g[:, dh, :], in_=ps, func=AF.Gelu,
                                         bias=b1_16[:, dh:dh + 1])
                for sub in range(4):
                    mtg = tch * 4 + sub
                    ps = psA.tile([P, D], F32, tag="psA")
                    for dh in range(NDH):
                        nc.tensor.matmul(
                            ps, lhsT=H1g[:, dh, sub * P:(sub + 1) * P],
                            rhs=W2s[:, dh, :], start=(dh == 0), stop=(dh == NDH - 1))
                    y3 = outp.tile([P, D], F32, tag="y3")
                    nc.vector.tensor_tensor(out=y3, in0=ps, in1=y2[:, mtg, :], op=OP.add)
                    y3b = y3
                    nc.gpsimd.tensor_tensor(out=y3b, in0=y3, in1=b2k_b, op=OP.add)
                    # final LN_s: only subtract the mean on device; the
                    # variance is shipped out and the host divides by
                    # sqrt(var+eps) (and applies gain/bias) — keeps Sqrt (a
                    # conflicting act table) out of the Gelu region.
                    st = small.tile([P, 6], F32)
                    nc.vector.bn_stats(out=st, in_=y3b)
                    mv = small.tile([P, 2], F32)
                    nc.vector.bn_aggr(out=mv, in_=st)
                    nc.vector.tensor_copy(out=var_sb[:, mtg:mtg + 1],
                                          in_=mv[:, 1:2])
                    o_sb = outp.tile([P, D], F32, tag="o_sb")
                    nc.vector.tensor_scalar_sub(out=o_sb, in0=y3b,
                                                scalar1=mv[:, 0:1])
                    nc.sync.dma_start(out=out_t[mtg], in_=o_sb)
            nc.sync.dma_start(out=var_d[:, :], in_=var_sb)

        _body()
    nc.finalize()
    return nc


_PROG_CACHE = {}


def kernel(**inputs) -> np.ndarray:
    f32 = np.float32
    bf = ml_dtypes.bfloat16
    x_m = np.asarray(inputs["x_m"], f32)
    A = np.asarray(inputs["A"], f32)
    g = {k: np.asarray(v, f32) for k, v in inputs.items()}

    # fold LN affine params into following matmuls (exact algebra)
    Wq = g["ln_q_g"][:, None] * g["Wq"]
    bq = g["bq"] + g["ln_q_b"] @ g["Wq"]
    Wk = g["ln_kv_g"][:, None] * g["Wk"]
    bk = g["bk"] + g["ln_kv_b"] @ g["Wk"]
    Wv = g["ln_kv_g"][:, None] * g["Wv"]
    bv = g["bv"] + g["ln_kv_b"] @ g["Wv"]
    W1 = g["ln_f_g"][:, None] * g["W1"]
    b1 = g["b1"] + g["ln_f_b"] @ g["W1"]

    add_bo = bool(np.any(g["bo"] != 0.0))
    add_bin = bool(np.any(g["b_in"] != 0.0))
    add_bv = bool(np.any(bv != 0.0))
    key = (add_bo, add_bin, add_bv)
    if key not in _PROG_CACHE:
        _PROG_CACHE[key] = build_program(add_bo, add_bin=add_bin, add_bv=add_bv)
    nc = _PROG_CACHE[key]

    common = {
        "Win": np.ascontiguousarray(g["W_in"].astype(bf)),
        "Wq": np.ascontiguousarray(Wq.astype(bf)),
        "Wk": np.ascontiguousarray(Wk.astype(bf)),
        "Wv": np.ascontiguousarray(Wv.astype(bf)),
        "Wo": np.ascontiguousarray(g["Wo"].astype(bf)),
        "W1": np.ascontiguousarray(W1.astype(bf)),
        "W2": np.ascontiguousarray(g["W2"].astype(bf)),
    }
    in_maps = []
    for c in range(8):
        b, k = c // 2, c % 2
        im = dict(common)
        im["xmT"] = np.ascontiguousarray(x_m[b].T.astype(bf))
        sm = np.zeros((128, 36), f32)
        sm[:, 0:12] = A[b, :, k].reshape(12, 128).T
        sm[:, 12:16] = bq.reshape(4, 128).T
        sm[:, 16:20] = bk.reshape(4, 128).T
        sm[:, 20:36] = b1.reshape(16, 128).T
        im["smalls"] = sm
        rows = np.stack([g["b_in"], bv, g["b2"] + g["spk_tags"][k], g["bo"]])
        im["rows"] = rows.astype(f32)
        in_maps.append(im)

    res = run_bass_kernel_spmd(nc, in_maps, core_ids=list(range(8)))
    out = np.zeros((B, KSP * T, D), f32)
    gs, bs = g["ln_s_g"], g["ln_s_b"]
    for c in range(8):
        b, k = c // 2, c % 2
        # device output is (y3 - mean); finish LN_s here: divide by
        # sqrt(var+eps) (var shipped as [128, NT], col-major tiles), then
        # apply gain/bias.
        var = res.results[c]["var"]          # [128, NT]
        rstd = 1.0 / np.sqrt(var.T.reshape(T, 1) + 1e-5)
        out[b, k * T:(k + 1) * T] = res.results[c]["out"] * rstd * gs + bs
    return out

